# revision 1
# baseline (speedup 1.0000x reference)
import sys

sys.path.insert(0, "/opt/trn_rl_repo")

import numpy as np

import concourse.bass as bass
import concourse.tile as tile
from concourse import mybir
from concourse.bass_utils import run_bass_kernel_spmd

# Problem constants (nn_DeltaNet_31877247271467)
B, L, HS = 4, 4096, 1024
NH, DK, DV = 4, 256, 256
CONV, CHUNK, FIRS, FIRL = 4, 32, 5, 64
DECAY = 1.0 - 1.0 / 3000.0
EPS_FLOOR = 0.08 * DECAY
RMS_EPS = 1e-05

FH = 2 * DK  # 512 features per head-half (2 heads of 256)
LB = 512     # L block for device matmul
KO = HS // 128  # 8 contraction tiles


def _build_nc():
    """Per-core SPMD program: qT/kT/vT = W{q,k,v}T_half.T-style projections.

    Inputs (per core): hT (HS, L) = hidden[b].T, w{q,k,v}T (HS, FH) =
    W{q,k,v}[head_half_rows].T.  Outputs: {q,k,v}T (FH, L).
    Contraction over HS on the partition dim, fp32r matmuls (full rate at
    free dim 512), accumulated in PSUM over 8 K-tiles.
    """
    nc = bass.Bass()
    f32 = mybir.dt.float32
    LTOT = L + 3 * FH  # hidden columns then wq|wk|wv weight columns
    X = nc.declare_dram_parameter("X", [HS, LTOT], f32, isOutput=False)
    wouts = {}
    for n in ("q", "k", "v"):
        wouts[n] = nc.declare_dram_parameter(f"{n}T", [FH, L], f32, isOutput=True)

    groups = []
    for lb in range(L // LB):
        for ni, n in enumerate(("q", "k", "v")):
            for m in range(FH // 128):
                groups.append((lb, ni, n, m))
    NG = len(groups)

    with (
        nc.sbuf_tensor([128, KO, LTOT], f32) as xt,
        nc.sbuf_tensor([128, 2, LB], f32) as ob,
        nc.psum_tensor([128, 2, LB], f32) as psum,
        nc.semaphore("dsem") as dsem,
        nc.semaphore("psem") as psem,
        nc.semaphore("vsem") as vsem,
        nc.semaphore("osem") as osem,
        nc.Block() as block,
    ):

        @block.gpsimd
        def _(gps):
            gps.dma_start(
                out=xt[:, :, :], in_=X.rearrange("(ko p) n -> p ko n", p=128)
            ).then_inc(dsem, 16)
            for g, (lb, ni, n, m) in enumerate(groups):
                gps.wait_ge(vsem, g + 1)
                gps.dma_start(
                    out=wouts[n][m * 128 : (m + 1) * 128, lb * LB : (lb + 1) * LB],
                    in_=ob[:, g % 2, :],
                ).then_inc(osem, 16)

        @block.tensor
        def _(pe):
            pe.wait_ge(dsem, 16)
            for g, (lb, ni, n, m) in enumerate(groups):
                wcol = L + ni * FH + m * 128
                if g >= 2:
                    pe.wait_ge(vsem, g - 1)
                for k in range(KO):
                    ins = pe.matmul(
                        psum[:, g % 2, :],
                        xt[:, k, wcol : wcol + 128],
                        xt[:, k, lb * LB : (lb + 1) * LB],
                        start=(k == 0),
                        stop=(k == KO - 1),
                    )
                    if k == KO - 1:
                        ins.then_inc(psem, 1)

        @block.vector
        def _(vec):
            for g in range(NG):
                vec.wait_ge(psem, g + 1)
                if g >= 2:
                    vec.wait_ge(osem, (g - 1) * 16)
                vec.tensor_copy(out=ob[:, g % 2, :], in_=psum[:, g % 2, :]).then_inc(
                    vsem, 1
                )

    return nc


def _dwconv_causal(x, filt):
    # x: (b, l, ch), filt: (ch, K) depthwise causal FIR
    K = filt.shape[-1]
    b, l, ch = x.shape
    y = np.zeros_like(x)
    for k in range(K):
        shift = K - 1 - k  # tap k reads x[t - shift]
        if shift == 0:
            y += filt[:, k] * x
        else:
            y[:, shift:, :] += filt[:, k] * x[:, :-shift, :]
    return y


def _silu(x):
    return x / (1.0 + np.exp(-x)) * np.ones((), np.float32)


def _sigmoid(x):
    return 1.0 / (1.0 + np.exp(-x))


def _gelu_tanh(x):
    c = np.float32(np.sqrt(2.0 / np.pi))
    return 0.5 * x * (1.0 + np.tanh(c * (x + 0.044715 * x * x * x)))


def _l2norm(x):
    return x / np.sqrt(np.sum(x * x, -1, keepdims=True) + 1e-6)


def _delta_rule_chunkwise(q, k, v, beta, chunk=CHUNK):
    b, h, Lq, dk = q.shape
    dv = v.shape[-1]
    n = Lq // chunk
    q = _l2norm(q).astype(np.float32)
    k = _l2norm(k).astype(np.float32)
    v = (v * beta[..., None]).astype(np.float32)
    kb = (k * beta[..., None]).astype(np.float32)
    r = lambda x: x.reshape(b, h, n, chunk, dv if x.shape[-1] == dv else dk)
    q, k, v, kb = r(q), r(k), r(v), r(kb)
    strict_low = np.tril(np.ones((chunk, chunk), bool), -1)
    A = np.where(strict_low, -np.einsum("bhnid,bhnjd->bhnij", kb, k), 0.0).astype(
        np.float32
    )
    eye = np.eye(chunk, dtype=np.float64)
    T = np.linalg.inv(eye - A.astype(np.float64)).astype(np.float32)
    u = T @ v
    w = T @ kb
    low = np.tril(np.ones((chunk, chunk), bool))
    S = np.zeros((b, h, dk, dv), np.float32)
    o = np.empty((b, h, n, chunk, dv), np.float32)
    for i in range(n):
        qi, ki, ui, wi = q[:, :, i], k[:, :, i], u[:, :, i], w[:, :, i]
        attn = np.where(low, np.einsum("bhid,bhjd->bhij", qi, ki), 0.0).astype(
            np.float32
        )
        u_i = ui - wi @ S
        o[:, :, i] = qi @ S + attn @ u_i
        S = S + np.einsum("bhcd,bhce->bhde", ki, u_i)
    return o.reshape(b, h, Lq, dv)


def _stats(x):
    mean = np.mean(x, -1, keepdims=True)
    var = np.var(x, -1, keepdims=True)
    am = np.mean(np.abs(x), -1, keepdims=True)
    l2 = np.sqrt(np.sum(x * x, -1, keepdims=True))
    return np.concatenate([mean, var, am, l2], -1).astype(np.float32)


def kernel(
    hidden_states,
    Wq,
    Wk,
    Wv,
    Wb,
    conv_q_w,
    conv_k_w,
    conv_v_w,
    fir_short_filt,
    fir_long_filt,
    gate_W1,
    gate_b1,
    gate_W2,
    gate_b2,
    gate_copy_bias,
    gate_log_temp,
    o_norm_w,
    Wo,
):
    hidden_states = np.asarray(hidden_states, np.float32)
    b, l, _ = hidden_states.shape

    # ---- device: q/k/v projections, sharded over (batch, head-half) on 8 cores
    nc = _build_nc()
    hT = np.ascontiguousarray(hidden_states.transpose(0, 2, 1))  # (B, HS, L)
    in_maps = []
    for c in range(8):
        bb, hg = c // 2, c % 2
        rows = slice(hg * FH, (hg + 1) * FH)
        X = np.concatenate(
            [
                hT[bb],
                np.asarray(Wq, np.float32)[rows].T,
                np.asarray(Wk, np.float32)[rows].T,
                np.asarray(Wv, np.float32)[rows].T,
            ],
            axis=1,
        )
        in_maps.append({"X": np.ascontiguousarray(X)})
    res = run_bass_kernel_spmd(nc, in_maps, list(range(8))).results

    def gather(name):
        out = np.empty((B, l, NH * DK), np.float32)
        for c in range(8):
            bb, hg = c // 2, c % 2
            out[bb, :, hg * FH : (hg + 1) * FH] = np.asarray(res[c][name]).T
        return out

    q_pre, k_pre, v_pre = gather("qT"), gather("kT"), gather("vT")

    # ---- host: the rest of the module in fp32 numpy
    q = _silu(_dwconv_causal(q_pre, np.asarray(conv_q_w, np.float32)))
    k = _silu(_dwconv_causal(k_pre, np.asarray(conv_k_w, np.float32)))
    v = _silu(_dwconv_causal(v_pre, np.asarray(conv_v_w, np.float32)))
    beta = _sigmoid(hidden_states @ np.asarray(Wb, np.float32).T)  # (b,l,h)

    qh = q.reshape(b, l, NH, DK).transpose(0, 2, 1, 3)
    kh = k.reshape(b, l, NH, DK).transpose(0, 2, 1, 3)
    vh = v.reshape(b, l, NH, DV).transpose(0, 2, 1, 3)
    o_d = _delta_rule_chunkwise(qh, kh, vh, beta.transpose(0, 2, 1))
    o_d = o_d.transpose(0, 2, 1, 3)  # (b,l,h,dv)

    v_direct = v.reshape(b, l, NH, DV)
    vc = v_direct.reshape(b, l, NH * DV)
    fir_s = _dwconv_causal(
        vc, np.asarray(fir_short_filt, np.float32).reshape(NH * DV, FIRS)
    ).reshape(b, l, NH, DV)
    fir_l = _dwconv_causal(
        vc, np.asarray(fir_long_filt, np.float32).reshape(NH * DV, FIRL)
    ).reshape(b, l, NH, DV)

    stats = np.concatenate(
        [_stats(fir_s), _stats(fir_l), _stats(o_d), _stats(v_direct)], -1
    )
    gin = np.concatenate(
        [np.broadcast_to(hidden_states[:, :, None, :], (b, l, NH, HS)), stats], -1
    ).astype(np.float32)
    h1 = _gelu_tanh(gin @ np.asarray(gate_W1, np.float32).T + np.asarray(gate_b1, np.float32))
    logits = h1 @ np.asarray(gate_W2, np.float32).T + np.asarray(gate_b2, np.float32)
    bias_val = np.asarray(gate_copy_bias, np.float32) * DECAY
    logits = logits + bias_val[None, None, :, None] * np.array(
        [0.0, 0.0, 0.0, 1.0], np.float32
    )
    temp = np.exp(np.asarray(gate_log_temp, np.float32))
    z = logits / temp[None, None, :, None]
    z = z - z.max(-1, keepdims=True)
    ez = np.exp(z)
    wgt = ez / ez.sum(-1, keepdims=True)
    wgt = wgt * (1.0 - 4.0 * EPS_FLOOR) + EPS_FLOOR
    o = (
        wgt[..., 0:1] * fir_s
        + wgt[..., 1:2] * fir_l
        + wgt[..., 2:3] * o_d
        + wgt[..., 3:4] * v_direct
    )
    o = (
        o
        / np.sqrt(np.mean(o * o, -1, keepdims=True) + RMS_EPS)
        * np.asarray(o_norm_w, np.float32)
    )
    return (o.reshape(b, l, NH * DV) @ np.asarray(Wo, np.float32).T).astype(np.float32)



# revision 2
# speedup vs baseline: 29.4024x; 29.4024x over previous
"""nn_DeltaNet_31877247271467 — fully-fused TRN2 Bass kernel (8 NeuronCores).

Sharding: core c = (batch c//2, head-half c%2).  hidden/output move host<->device
as fp16; all device compute is fp32 (fp16 operands feed the big matmuls).
The forward pass runs as 5 small chained NEFFs with device-resident
intermediates (AllGather pairs for hidden, ReduceScatter pairs for the output
projection partials).  NEFFs are built, compiled, and warm-loaded at import.
"""

import sys

sys.path.insert(0, "/opt/trn_rl_repo")

import numpy as np
from contextlib import ExitStack

import jax
import concourse.bass as bass
import concourse.bacc as bacc
import concourse.tile as tile
from concourse import mybir
from concourse.bass2jax import _bass_exec_p, install_neuronx_cc_hook, partition_id_tensor
from jax.experimental.shard_map import shard_map
from jax.sharding import Mesh, PartitionSpec

f32, f16, bf16 = mybir.dt.float32, mybir.dt.float16, mybir.dt.bfloat16
AF = mybir.ActivationFunctionType
OP = mybir.AluOpType

B, L, HS = 4, 4096, 1024
NH, DK, DV = 4, 256, 256
CONV, FIRS, FIRL = 4, 5, 64
DECAY = 1.0 - 1.0 / 3000.0
EPS_FLOOR = 0.08 * DECAY
RMS_EPS = 1e-05
CH = 512
NB = L // 512
PAIRS = [[0, 1], [2, 3], [4, 5], [6, 7]]

WPK1 = [("wqT", (HS, CH)), ("wkT", (HS, CH)), ("wvT", (HS, CH)), ("wbT", (HS, 2))]
CPK1 = [("convq", (CH, CONV)), ("convk", (CH, CONV)), ("convv", (CH, CONV)),
        ("firs", (CH, FIRS)), ("firl", (CH, FIRL))]
CPK2 = [("iden", (128, 128)), ("negU", (128, 128)), ("uincl", (128, 128)),
        ("ones", (128, 128))]
WPK3 = [("w1hT", (HS, HS)), ("woT", (CH, HS))]
CPK3 = [("ones", (128, 128)), ("w1sT", (16, HS)), ("w2T", (HS, 4)),
        ("b1", (HS, 1)), ("normw", (DV, 1)), ("b2s", (4, 2)), ("invt", (4, 2))]


def _offsets(spec):
    offs, o = {}, 0
    for name, shp in spec:
        offs[name] = o
        o += int(np.prod(shp))
    return offs, o


W1OFF, W1TOT = _offsets(WPK1)
C1OFF, C1TOT = _offsets(CPK1)
C2OFF, C2TOT = _offsets(CPK2)
W3OFF, W3TOT = _offsets(WPK3)
C3OFF, C3TOT = _offsets(CPK3)


def _ap(dram_t, off, ap):
    return bass.AP(tensor=dram_t[0, :].tensor, offset=off, ap=ap)


class KB:
    """Shared per-kernel build helpers."""

    def __init__(self):
        self.nc = bacc.Bacc("TRN2", target_bir_lowering=False, debug=False,
                            num_devices=8)

    def start(self, tc):
        self.tc = tc
        self.ctx = ExitStack()
        self.const = self.ctx.enter_context(tc.tile_pool(name="const", bufs=1))
        self.dram = self.ctx.enter_context(
            tc.tile_pool(name="dram", bufs=1, space="DRAM"))
        self.ps = self.ctx.enter_context(
            tc.tile_pool(name="ps", bufs=7, space="PSUM"))
        self.sc = self.ctx.enter_context(tc.tile_pool(name="sc", bufs=10))

    def psum(self, shape):
        return self.ps.tile(shape, f32, tag="pp", name="pp")

    def scratch(self, shape, dt=f32):
        return self.sc.tile(shape, dt, tag="s512", name="s512")

    def ld2(self, src_dram, name, shape, off, dt=f32, parts=None):
        p = parts or shape[0]
        t = self.const.tile([p, shape[1]], dt, tag="c_" + name, name="c_" + name)
        self.nc.sync.dma_start(out=t[:shape[0], :],
                               in_=_ap(src_dram, off,
                                       [[shape[1], shape[0]], [1, shape[1]]]))
        return t

    def ld3(self, src_dram, name, ktiles, inner, off, dt=f32):
        t = self.const.tile([128, ktiles, inner], dt, tag="c3_" + name,
                            name="c3_" + name)
        self.nc.sync.dma_start(
            out=t[:, :, :],
            in_=_ap(src_dram, off,
                    [[inner, 128], [128 * inner, ktiles], [1, inner]]))
        return t


# ================= K1 =================
def build_k1():
    kb = KB()
    nc = kb.nc
    HH = nc.dram_tensor("hhalf", [L // 2, HS], f16, kind="ExternalInput")
    WP = nc.dram_tensor("wpk1", [1, W1TOT], f16, kind="ExternalInput")
    CP = nc.dram_tensor("cpk1", [1, C1TOT], f32, kind="ExternalInput")
    HID16 = nc.dram_tensor("hid16", [L, HS], f16, kind="ExternalOutput")
    QC = nc.dram_tensor("qc", [CH, L], f32, kind="ExternalOutput")
    KC = nc.dram_tensor("kc", [CH, L], f32, kind="ExternalOutput")
    VC = nc.dram_tensor("vc", [CH, L + 63], f32, kind="ExternalOutput")
    FS = nc.dram_tensor("fs", [CH, L], f32, kind="ExternalOutput")
    FL = nc.dram_tensor("fl", [CH, L], f32, kind="ExternalOutput")
    BETA = nc.dram_tensor("beta", [2, L], f32, kind="ExternalOutput")
    with tile.TileContext(nc) as tc:
        kb.start(tc)
        wq = kb.ld3(WP, "wqT", 8, CH, W1OFF["wqT"], f16)
        wk = kb.ld3(WP, "wkT", 8, CH, W1OFF["wkT"], f16)
        wv = kb.ld3(WP, "wvT", 8, CH, W1OFF["wvT"], f16)
        wb = kb.ld3(WP, "wbT", 8, 2, W1OFF["wbT"], f16)
        conv_t = {n: kb.ld3(CP, "conv" + n, 4, CONV, C1OFF["conv" + n])
                  for n in "qkv"}
        firs_t = kb.ld3(CP, "firs", 4, FIRS, C1OFF["firs"])
        firl_t = kb.ld3(CP, "firl", 4, FIRL, C1OFF["firl"])
        zpad = kb.const.tile([128, 64], f32)
        nc.vector.memset(zpad[:, :], 0.0)

        hid_half = kb.dram.tile([L // 2, HS], f16)
        hid = kb.dram.tile([L, HS], f16)
        raw = {n: kb.dram.tile([CH, L + 3], f32, tag="raw" + n, name="raw" + n)
               for n in "qkv"}
        nc.gpsimd.dma_start(hid_half[:, :], HH[:, :])
        nc.gpsimd.collective_compute(
            "AllGather", OP.bypass, replica_groups=PAIRS,
            ins=[hid_half.opt()], outs=[hid.opt()])
        nc.gpsimd.dma_start(HID16[:, :], hid[:, :])
        hidT = hid[:, :].rearrange("a b -> b a")

        for n in "qkv":
            for m in range(4):
                nc.sync.dma_start(out=raw[n][128 * m:128 * (m + 1), 0:3],
                                  in_=zpad[:, 0:3])
        for m in range(4):
            nc.sync.dma_start(out=VC[128 * m:128 * (m + 1), 0:63],
                              in_=zpad[:, 0:63])

        with tc.tile_pool(name="p1", bufs=2) as p1:
            for tb in range(NB):
                t0 = tb * 512
                ht = []
                for k in range(8):
                    h = p1.tile([128, 512], f16, tag=f"ht{k}", name=f"ht{k}")
                    nc.sync.dma_start(
                        out=h[:, :], in_=hidT[128 * k:128 * (k + 1), t0:t0 + 512])
                    ht.append(h)
                for (wt, dst) in ((wq, raw["q"]), (wk, raw["k"]), (wv, raw["v"])):
                    for m in range(4):
                        p = kb.psum([128, 512])
                        for k in range(8):
                            nc.tensor.matmul(p[:, :],
                                             wt[:, k, 128 * m:128 * (m + 1)],
                                             ht[k][:, :], start=(k == 0),
                                             stop=(k == 7))
                        sb = kb.scratch([128, 512])
                        nc.scalar.copy(out=sb[:, :], in_=p[:, :])
                        nc.sync.dma_start(
                            out=dst[128 * m:128 * (m + 1), 3 + t0:3 + t0 + 512],
                            in_=sb[:, :])
                pb = kb.psum([2, 512])
                for k in range(8):
                    nc.tensor.matmul(pb[:, :], wb[:, k, :], ht[k][:, :],
                                     start=(k == 0), stop=(k == 7))
                bsg = kb.scratch([2, 512])
                nc.scalar.activation(out=bsg[:, :], in_=pb[:, :], func=AF.Sigmoid)
                nc.sync.dma_start(out=BETA[:, t0:t0 + 512], in_=bsg[:, :])

        # conv + silu, full width
        with tc.tile_pool(name="p2", bufs=2) as p2:
            for n, dst, voff in (("q", QC, 0), ("k", KC, 0), ("v", VC, 63)):
                for m in range(4):
                    w = p2.tile([128, L + 3], f32, tag="convw", name="convw")
                    nc.sync.dma_start(out=w[:, :],
                                      in_=raw[n][128 * m:128 * (m + 1), :])
                    acc = p2.tile([128, L], f32, tag="convacc", name="convacc")
                    nc.vector.tensor_scalar(out=acc[:, :], in0=w[:, 0:L],
                                            scalar1=conv_t[n][:, m, 0:1],
                                            scalar2=None, op0=OP.mult)
                    for j in range(1, 4):
                        nc.vector.scalar_tensor_tensor(
                            out=acc[:, :], in0=w[:, j:j + L],
                            scalar=conv_t[n][:, m, j:j + 1], in1=acc[:, :],
                            op0=OP.mult, op1=OP.add)
                    sb = p2.tile([128, L], f32, tag="convout", name="convout")
                    nc.scalar.activation(out=sb[:, :], in_=acc[:, :], func=AF.Silu)
                    nc.sync.dma_start(
                        out=dst[128 * m:128 * (m + 1), voff:voff + L], in_=sb[:, :])
        # FIR, full width
        with tc.tile_pool(name="p3", bufs=2) as p3:
            for m in range(4):
                w = p3.tile([128, L + 63], f32, tag="firw", name="firw")
                nc.sync.dma_start(out=w[:, :], in_=VC[128 * m:128 * (m + 1), :])
                accl = p3.tile([128, L], f32, tag="firaccl", name="firaccl")
                nc.vector.tensor_scalar(out=accl[:, :], in0=w[:, 0:L],
                                        scalar1=firl_t[:, m, 0:1], scalar2=None,
                                        op0=OP.mult)
                for j in range(1, FIRL):
                    nc.vector.scalar_tensor_tensor(
                        out=accl[:, :], in0=w[:, j:j + L],
                        scalar=firl_t[:, m, j:j + 1], in1=accl[:, :],
                        op0=OP.mult, op1=OP.add)
                nc.sync.dma_start(out=FL[128 * m:128 * (m + 1), :], in_=accl[:, :])
                accs = p3.tile([128, L], f32, tag="firaccs", name="firaccs")
                nc.vector.tensor_scalar(out=accs[:, :], in0=w[:, 59:59 + L],
                                        scalar1=firs_t[:, m, 0:1], scalar2=None,
                                        op0=OP.mult)
                for j in range(1, FIRS):
                    nc.vector.scalar_tensor_tensor(
                        out=accs[:, :], in0=w[:, 59 + j:59 + j + L],
                        scalar=firs_t[:, m, j:j + 1], in1=accs[:, :],
                        op0=OP.mult, op1=OP.add)
                nc.sync.dma_start(out=FS[128 * m:128 * (m + 1), :], in_=accs[:, :])
        kb.ctx.close()
    nc.compile()
    return nc


# ================= K2 (per local head) =================
def build_k2(h):
    kb = KB()
    nc = kb.nc
    QC = nc.dram_tensor("qc", [CH, L], f32, kind="ExternalInput")
    KC = nc.dram_tensor("kc", [CH, L], f32, kind="ExternalInput")
    VC = nc.dram_tensor("vc", [CH, L + 63], f32, kind="ExternalInput")
    BETA = nc.dram_tensor("beta", [2, L], f32, kind="ExternalInput")
    CP = nc.dram_tensor("cpk2", [1, C2TOT], f32, kind="ExternalInput")
    ODC = nc.dram_tensor(f"odc{h}", [256, L], f32, kind="ExternalOutput")
    r0 = 256 * h
    with tile.TileContext(nc) as tc:
        kb.start(tc)
        iden = kb.ld2(CP, "iden", (128, 128), C2OFF["iden"])
        negU = kb.ld2(CP, "negU", (128, 128), C2OFF["negU"])
        uincl = kb.ld2(CP, "uincl", (128, 128), C2OFF["uincl"])
        ones = kb.ld2(CP, "ones", (128, 128), C2OFF["ones"])
        epsl2 = kb.const.tile([128, 1], f32)
        nc.vector.memset(epsl2[:, :], 1e-6)

        with tc.tile_pool(name="sp", bufs=1) as sp, \
             tc.tile_pool(name="stash", bufs=3) as stash, \
             tc.tile_pool(name="qkv", bufs=2) as qkv:
            S_sb = [sp.tile([128, 256], f32, tag=f"S{j}", name=f"S{j}")
                    for j in range(2)]
            for j in range(2):
                nc.vector.memset(S_sb[j][:, :], 0.0)
            for blk in range(NB):
                t0 = blk * 512
                qd, kd, vd = [], [], []
                for j in range(2):
                    rj = r0 + 128 * j
                    q_ = qkv.tile([128, 512], f32, tag=f"qd{j}", name=f"qd{j}")
                    nc.sync.dma_start(out=q_[:, :], in_=QC[rj:rj + 128, t0:t0 + 512])
                    qd.append(q_)
                    k_ = qkv.tile([128, 512], f32, tag=f"kd{j}", name=f"kd{j}")
                    nc.sync.dma_start(out=k_[:, :], in_=KC[rj:rj + 128, t0:t0 + 512])
                    kd.append(k_)
                    v_ = qkv.tile([128, 512], f32, tag=f"vd{j}", name=f"vd{j}")
                    nc.sync.dma_start(out=v_[:, :],
                                      in_=VC[rj:rj + 128, 63 + t0:63 + t0 + 512])
                    vd.append(v_)
                bb = qkv.tile([1, 512], f32, tag="bb", name="bb")
                nc.sync.dma_start(out=bb[:, :], in_=BETA[h:h + 1, t0:t0 + 512])
                rq_b = qkv.tile([1, 512], f32, tag="rqb", name="rqb")
                rk_b = qkv.tile([1, 512], f32, tag="rkb", name="rkb")
                for (dsrc, rdst) in ((qd, rq_b), (kd, rk_b)):
                    sqt = [kb.scratch([128, 512]) for _ in range(2)]
                    for j in range(2):
                        nc.scalar.activation(out=sqt[j][:, :], in_=dsrc[j][:, :],
                                             func=AF.Square)
                    pssum = kb.psum([1, 512])
                    for j in range(2):
                        nc.tensor.matmul(pssum[:, :], ones[:, 0:1], sqt[j][:, :],
                                         start=(j == 0), stop=(j == 1))
                    nrm = kb.scratch([1, 512])
                    nc.scalar.activation(out=nrm[:, :], in_=pssum[:, :],
                                         func=AF.Sqrt, bias=epsl2[0:1, :])
                    nc.vector.reciprocal(out=rdst[:, :], in_=nrm[:, :])
                for cc in range(4):
                    c0 = t0 + cc * 128
                    s0 = cc * 128
                    pcol = kb.psum([128, 3])
                    nc.tensor.matmul(pcol[:, 0:1], bb[0:1, s0:s0 + 128],
                                     ones[0:1, 0:1], start=True, stop=True)
                    nc.tensor.matmul(pcol[:, 1:2], rq_b[0:1, s0:s0 + 128],
                                     ones[0:1, 0:1], start=True, stop=True)
                    nc.tensor.matmul(pcol[:, 2:3], rk_b[0:1, s0:s0 + 128],
                                     ones[0:1, 0:1], start=True, stop=True)
                    cols = stash.tile([128, 3], f32, tag="cols", name="cols")
                    nc.vector.tensor_copy(out=cols[:, :], in_=pcol[:, :])
                    bcol, rqcol, rkcol = cols[:, 0:1], cols[:, 1:2], cols[:, 2:3]
                    brk = kb.scratch([1, 128])
                    nc.vector.tensor_tensor(out=brk[:, :],
                                            in0=bb[0:1, s0:s0 + 128],
                                            in1=rk_b[0:1, s0:s0 + 128], op=OP.mult)
                    prb = kb.psum([128, 128])
                    nc.tensor.matmul(prb[:, :], ones[0:1, :], brk[:, :],
                                     start=True, stop=True)
                    pg = kb.psum([128, 128])
                    for j in range(2):
                        nc.tensor.matmul(pg[:, :], kd[j][:, s0:s0 + 128],
                                         kd[j][:, s0:s0 + 128], start=(j == 0),
                                         stop=(j == 1))
                    ptmp = kb.scratch([128, 128])
                    nc.vector.scalar_tensor_tensor(out=ptmp[:, :], in0=pg[:, :],
                                                   scalar=rkcol, in1=negU[:, :],
                                                   op0=OP.mult, op1=OP.mult)
                    P_sb = kb.scratch([128, 128])
                    nc.vector.tensor_tensor(out=P_sb[:, :], in0=ptmp[:, :],
                                            in1=prb[:, :], op=OP.mult)
                    pa = kb.psum([128, 128])
                    for j in range(2):
                        nc.tensor.matmul(pa[:, :], kd[j][:, s0:s0 + 128],
                                         qd[j][:, s0:s0 + 128], start=(j == 0),
                                         stop=(j == 1))
                    attnT = stash.tile([128, 128], f32, tag="attnT", name="attnT")
                    nc.vector.scalar_tensor_tensor(out=attnT[:, :], in0=pa[:, :],
                                                   scalar=rkcol, in1=uincl[:, :],
                                                   op0=OP.mult, op1=OP.mult)
                    pkt = kb.psum([128, 256])
                    pvt = kb.psum([128, 256])
                    for j in range(2):
                        nc.tensor.matmul(pkt[:, 128 * j:128 * (j + 1)],
                                         kd[j][:, s0:s0 + 128], iden[:, :],
                                         start=True, stop=True)
                        nc.tensor.matmul(pvt[:, 128 * j:128 * (j + 1)],
                                         vd[j][:, s0:s0 + 128], iden[:, :],
                                         start=True, stop=True)
                    kntm = stash.tile([128, 256], f32, tag="kntm", name="kntm")
                    nc.vector.tensor_scalar(out=kntm[:, :], in0=pkt[:, :],
                                            scalar1=rkcol, scalar2=None,
                                            op0=OP.mult)
                    uw = stash.tile([128, 512], f32, tag="uw", name="uw")
                    nc.vector.tensor_scalar(out=uw[:, 0:256], in0=pvt[:, :],
                                            scalar1=bcol, scalar2=None,
                                            op0=OP.mult)
                    nc.vector.tensor_scalar(out=uw[:, 256:512], in0=kntm[:, :],
                                            scalar1=bcol, scalar2=None,
                                            op0=OP.mult)
                    for lvl in range(7):
                        puw = kb.psum([128, 512])
                        nc.tensor.matmul(puw[:, :], P_sb[:, :], uw[:, :],
                                         start=True, stop=True)
                        nc.vector.tensor_tensor(out=uw[:, :], in0=puw[:, :],
                                                in1=uw[:, :], op=OP.add)
                        if lvl < 6:
                            ptr = kb.psum([128, 128])
                            nc.tensor.matmul(ptr[:, :], P_sb[:, :], iden[:, :],
                                             start=True, stop=True)
                            PT_sb = kb.scratch([128, 128])
                            nc.vector.tensor_copy(out=PT_sb[:, :], in_=ptr[:, :])
                            psq = kb.psum([128, 128])
                            nc.tensor.matmul(psq[:, :], PT_sb[:, :], P_sb[:, :],
                                             start=True, stop=True)
                            P_sb = kb.scratch([128, 128])
                            nc.vector.tensor_copy(out=P_sb[:, :], in_=psq[:, :])
                    pwt = kb.psum([128, 256])
                    for j in range(2):
                        nc.tensor.matmul(pwt[:, 128 * j:128 * (j + 1)],
                                         uw[:, 256 + 128 * j:256 + 128 * (j + 1)],
                                         iden[:, :], start=True, stop=True)
                    wtcm = stash.tile([128, 256], f32, tag="wtcm", name="wtcm")
                    nc.vector.tensor_copy(out=wtcm[:, :], in_=pwt[:, :])

                    pws = kb.psum([128, 256])
                    for j in range(2):
                        nc.tensor.matmul(pws[:, :], wtcm[:, 128 * j:128 * (j + 1)],
                                         S_sb[j][:, :], start=(j == 0),
                                         stop=(j == 1))
                    ui = stash.tile([128, 256], f32, tag="ui", name="ui")
                    nc.vector.tensor_tensor(out=ui[:, :], in0=uw[:, 0:256],
                                            in1=pws[:, :], op=OP.subtract)
                    po = kb.psum([128, 256])
                    for j in range(2):
                        nc.tensor.matmul(po[:, :], qd[j][:, s0:s0 + 128],
                                         S_sb[j][:, :], start=(j == 0), stop=False)
                    nc.tensor.matmul(po[:, :], attnT[:, :], ui[:, :],
                                     start=False, stop=True)
                    otm = kb.scratch([128, 256])
                    nc.vector.tensor_scalar(out=otm[:, :], in0=po[:, :],
                                            scalar1=rqcol, scalar2=None,
                                            op0=OP.mult)
                    for j in range(2):
                        pds = kb.psum([128, 256])
                        nc.tensor.matmul(pds[:, :], kntm[:, 128 * j:128 * (j + 1)],
                                         ui[:, :], start=True, stop=True)
                        nc.vector.tensor_tensor(out=S_sb[j][:, :],
                                                in0=S_sb[j][:, :],
                                                in1=pds[:, :], op=OP.add)
                    pot = kb.psum([128, 256])
                    for j in range(2):
                        nc.tensor.matmul(pot[:, 128 * j:128 * (j + 1)],
                                         otm[:, 128 * j:128 * (j + 1)],
                                         iden[:, :], start=True, stop=True)
                    osb = kb.scratch([128, 256])
                    nc.scalar.copy(out=osb[:, :], in_=pot[:, :])
                    for j in range(2):
                        nc.sync.dma_start(
                            out=ODC[128 * j:128 * (j + 1), c0:c0 + 128],
                            in_=osb[:, 128 * j:128 * (j + 1)])
        kb.ctx.close()
    nc.compile()
    return nc


# ================= K3 (per time half) =================
def build_k3(p):
    kb = KB()
    nc = kb.nc
    HID16 = nc.dram_tensor("hid16", [L, HS], f16, kind="ExternalInput")
    VC = nc.dram_tensor("vc", [CH, L + 63], f32, kind="ExternalInput")
    FS = nc.dram_tensor("fs", [CH, L], f32, kind="ExternalInput")
    FL = nc.dram_tensor("fl", [CH, L], f32, kind="ExternalInput")
    OD0 = nc.dram_tensor("odc0", [256, L], f32, kind="ExternalInput")
    OD1 = nc.dram_tensor("odc1", [256, L], f32, kind="ExternalInput")
    WP = nc.dram_tensor("wpk3", [1, W3TOT], f16, kind="ExternalInput")
    CP = nc.dram_tensor("cpk3", [1, C3TOT], f32, kind="ExternalInput")
    if p == 0:
        OUTP = nc.dram_tensor("outp0", [L // 2, HS], f32, kind="ExternalOutput")
    else:
        O0 = nc.dram_tensor("outp0", [L // 2, HS], f32, kind="ExternalInput")
        OUT = nc.dram_tensor("out", [L // 2, HS], f16, kind="ExternalOutput")
    with tile.TileContext(nc) as tc:
        kb.start(tc)
        if p == 1:
            outp_full = kb.dram.tile([L, HS], f32)
            outr = kb.dram.tile([L // 2, HS], f32)
            OUTP = outp_full[L // 2:L, :]
            nc.gpsimd.dma_start(outp_full[0:L // 2, :], O0[:, :])
        nc_ = nc
        ones = kb.ld2(CP, "ones", (128, 128), C3OFF["ones"])
        w1s = kb.ld2(CP, "w1sT", (16, HS), C3OFF["w1sT"], parts=32)
        b2s_t = kb.ld2(CP, "b2s", (4, 2), C3OFF["b2s"], parts=32)
        invt_t = kb.ld2(CP, "invt", (4, 2), C3OFF["invt"], parts=32)
        w2f = kb.ld3(CP, "w2T", 8, 4, C3OFF["w2T"])
        w2_t = kb.const.tile([128, 8, 4], bf16)
        nc.vector.tensor_copy(out=w2_t[:, :, :], in_=w2f[:, :, :])
        b1_t = kb.const.tile([128, 8], f32)
        nc.sync.dma_start(out=b1_t[:, :],
                          in_=_ap(CP, C3OFF["b1"], [[1, 128], [128, 8]]))
        normw_t = kb.const.tile([128, 2], f32)
        nc.sync.dma_start(out=normw_t[:, :],
                          in_=_ap(CP, C3OFF["normw"], [[1, 128], [128, 2]]))
        epsrms = kb.const.tile([128, 1], f32)
        nc.vector.memset(epsrms[:, :], RMS_EPS)
        w1 = kb.ld3(WP, "w1hT", 8, HS, W3OFF["w1hT"], f16)
        wo16 = kb.ld3(WP, "woT", 4, HS, W3OFF["woT"], f16)
        hidT = HID16[:, :].rearrange("a b -> b a")

        STATS_SRC = [("fs", FS, 0), ("fl", FL, 0), ("od", None, 0), ("v", VC, 63)]
        with tc.tile_pool(name="p7", bufs=2) as p7, \
             tc.tile_pool(name="p7one", bufs=1) as p7one, \
             tc.tile_pool(name="onp", bufs=2) as onp:
            for tbl in range(4):
                tb = 4 * p + tbl
                t0 = tb * 512
                tl0 = tbl * 512
                ht = []
                for k in range(8):
                    hh = p7.tile([128, 512], f16, tag=f"ht{k}", name=f"ht{k}")
                    nc.sync.dma_start(
                        out=hh[:, :], in_=hidT[128 * k:128 * (k + 1), t0:t0 + 512])
                    ht.append(hh)
                on_all = []
                for h in range(2):
                    r0 = 256 * h
                    odh = OD0 if h == 0 else OD1

                    def src_rows(ti, j):
                        nm, src, voff = STATS_SRC[ti]
                        if nm == "od":
                            return odh[128 * j:128 * (j + 1), t0:t0 + 512]
                        return src[r0 + 128 * j:r0 + 128 * (j + 1),
                                   voff + t0:voff + t0 + 512]

                    stats_blk = p7.tile([32, 512], f32, tag="statsblk",
                                        name="statsblk")
                    for ti in range(4):
                        bt = [p7.tile([128, 512], f32, tag=f"bt{j}", name=f"bt{j}")
                              for j in range(2)]
                        for j in range(2):
                            nc.sync.dma_start(out=bt[j][:, :], in_=src_rows(ti, j))
                        sq = [kb.scratch([128, 512]) for _ in range(2)]
                        ab = [kb.scratch([128, 512]) for _ in range(2)]
                        for j in range(2):
                            nc.scalar.activation(out=sq[j][:, :], in_=bt[j][:, :],
                                                 func=AF.Square)
                            nc.scalar.activation(out=ab[j][:, :], in_=bt[j][:, :],
                                                 func=AF.Abs)
                        psx = kb.psum([1, 512])
                        psq2 = kb.psum([1, 512])
                        psa = kb.psum([1, 512])
                        for j in range(2):
                            nc.tensor.matmul(psx[:, :], ones[:, 0:1], bt[j][:, :],
                                             start=(j == 0), stop=(j == 1))
                            nc.tensor.matmul(psq2[:, :], ones[:, 0:1], sq[j][:, :],
                                             start=(j == 0), stop=(j == 1))
                            nc.tensor.matmul(psa[:, :], ones[:, 0:1], ab[j][:, :],
                                             start=(j == 0), stop=(j == 1))
                        r = 4 * ti
                        fin = kb.scratch([1, 4, 512])
                        nc.scalar.activation(out=fin[:, 0, :], in_=psx[:, :],
                                             func=AF.Copy, scale=1.0 / 256.0)
                        m2 = kb.scratch([1, 512])
                        nc.vector.tensor_tensor(out=m2[:, :], in0=fin[:, 0, :],
                                                in1=fin[:, 0, :], op=OP.mult)
                        nc.vector.scalar_tensor_tensor(
                            out=fin[:, 1, :], in0=psq2[:, :], scalar=1.0 / 256.0,
                            in1=m2[:, :], op0=OP.mult, op1=OP.subtract)
                        nc.scalar.activation(out=fin[:, 2, :], in_=psa[:, :],
                                             func=AF.Copy, scale=1.0 / 256.0)
                        nc.scalar.activation(out=fin[:, 3, :], in_=psq2[:, :],
                                             func=AF.Sqrt)
                        nc.sync.dma_start(
                            out=stats_blk[r:r + 4, :],
                            in_=bass.AP(tensor=fin.tensor, offset=fin.offset,
                                        ap=[[512, 4], [1, 512]]))
                    # gate MLP
                    h1 = p7one.tile([128, 8, 512], bf16, tag="h1", name="h1")
                    for gm in range(8):
                        pg1 = kb.psum([128, 512])
                        for k in range(8):
                            nc.tensor.matmul(pg1[:, :],
                                             w1[:, k, 128 * gm:128 * (gm + 1)],
                                             ht[k][:, :], start=(k == 0),
                                             stop=False)
                        nc.tensor.matmul(pg1[:, :],
                                         w1s[0:16, 128 * gm:128 * (gm + 1)],
                                         stats_blk[0:16, :], start=False, stop=True)
                        nc.scalar.activation(out=h1[:, gm, :], in_=pg1[:, :],
                                             func=AF.Gelu_apprx_tanh,
                                             bias=b1_t[:, gm:gm + 1])
                    pl = kb.psum([4, 512])
                    for k in range(8):
                        nc.tensor.matmul(pl[:, :], w2_t[:, k, :], h1[:, k, :],
                                         start=(k == 0), stop=(k == 7))
                    el = kb.scratch([4, 512])
                    nc.scalar.activation(out=el[:, :], in_=pl[:, :], func=AF.Exp,
                                         scale=invt_t[0:4, h:h + 1],
                                         bias=b2s_t[0:4, h:h + 1])
                    pse = kb.psum([1, 512])
                    nc.tensor.matmul(pse[:, :], ones[0:4, 0:1], el[:, :],
                                     start=True, stop=True)
                    rec = kb.scratch([1, 512])
                    nc.vector.reciprocal(out=rec[:, :], in_=pse[:, :])
                    prr = kb.psum([4, 512])
                    nc.tensor.matmul(prr[:, :], ones[0:1, 0:4], rec[:, :],
                                     start=True, stop=True)
                    wgt = p7.tile([4, 512], f32, tag="wgtt", name="wgtt")
                    nc.vector.tensor_tensor(out=wgt[:, :], in0=el[:, :],
                                            in1=prr[:, :], op=OP.mult)
                    nc.vector.tensor_scalar(out=wgt[:, :], in0=wgt[:, :],
                                            scalar1=1.0 - 4.0 * EPS_FLOOR,
                                            scalar2=EPS_FLOOR, op0=OP.mult,
                                            op1=OP.add)
                    wbc = p7one.tile([128, 4, 512], f32, tag="wbc", name="wbc")
                    for cls in range(4):
                        wrow = kb.scratch([1, 512])
                        nc.sync.dma_start(out=wrow[:, :], in_=wgt[cls:cls + 1, :])
                        pwb = kb.psum([128, 512])
                        nc.tensor.matmul(pwb[:, :], ones[0:1, :], wrow[:, :],
                                         start=True, stop=True)
                        nc.scalar.copy(out=wbc[:, cls, :], in_=pwb[:, :])
                    ob = [p7one.tile([128, 512], f32, tag=f"ob{j}", name=f"ob{j}")
                          for j in range(2)]
                    osq = [kb.scratch([128, 512]) for _ in range(2)]
                    for cls in range(4):
                        for j in range(2):
                            bl = p7.tile([128, 512], f32, tag=f"bl{j}",
                                         name=f"bl{j}")
                            nc.sync.dma_start(out=bl[:, :], in_=src_rows(cls, j))
                            if cls == 0:
                                nc.vector.tensor_tensor(out=ob[j][:, :],
                                                        in0=bl[:, :],
                                                        in1=wbc[:, 0, :],
                                                        op=OP.mult)
                            else:
                                tbr = kb.scratch([128, 512])
                                nc.vector.tensor_tensor(out=tbr[:, :],
                                                        in0=bl[:, :],
                                                        in1=wbc[:, cls, :],
                                                        op=OP.mult)
                                nc.vector.tensor_tensor(out=ob[j][:, :],
                                                        in0=ob[j][:, :],
                                                        in1=tbr[:, :], op=OP.add)
                    for j in range(2):
                        nc.scalar.activation(out=osq[j][:, :], in_=ob[j][:, :],
                                             func=AF.Square)
                    pso = kb.psum([1, 512])
                    for j in range(2):
                        nc.tensor.matmul(pso[:, :], ones[:, 0:1], osq[j][:, :],
                                         start=(j == 0), stop=(j == 1))
                    srt = kb.scratch([1, 512])
                    nc.scalar.activation(out=srt[:, :], in_=pso[:, :],
                                         func=AF.Sqrt, scale=1.0 / 256.0,
                                         bias=epsrms[0:1, :])
                    rre = kb.scratch([1, 512])
                    nc.vector.reciprocal(out=rre[:, :], in_=srt[:, :])
                    prn = kb.psum([128, 512])
                    nc.tensor.matmul(prn[:, :], ones[0:1, :], rre[:, :],
                                     start=True, stop=True)
                    ons = []
                    for j in range(2):
                        on = onp.tile([128, 512], f16, tag=f"on{h}{j}",
                                      name=f"on{h}{j}")
                        nc.vector.scalar_tensor_tensor(
                            out=on[:, :], in0=ob[j][:, :],
                            scalar=normw_t[:, j:j + 1], in1=prn[:, :],
                            op0=OP.mult, op1=OP.mult)
                        ons.append(on)
                    on_all.append(ons)
                for m in range(8):
                    pw = kb.psum([128, 512])
                    for d in range(4):
                        nc.tensor.matmul(pw[:, :],
                                         wo16[:, d, 128 * m:128 * (m + 1)],
                                         on_all[d // 2][d % 2][:, :],
                                         start=(d == 0), stop=(d == 3))
                    owr = kb.scratch([128, 512])
                    nc.vector.tensor_copy(out=owr[:, :], in_=pw[:, :])
                    nc.sync.dma_start(
                        out=OUTP[tl0:tl0 + 512,
                                 128 * m:128 * (m + 1)].rearrange("a b -> b a"),
                        in_=owr[:, :])
        if p == 1:
            nc.gpsimd.collective_compute(
                "ReduceScatter", OP.add, replica_groups=PAIRS,
                ins=[outp_full.opt()], outs=[outr.opt()])
            with tc.tile_pool(name="p10", bufs=2) as p10:
                for r in range(16):
                    for half in range(2):
                        t = p10.tile([128, 512], f32, tag="oload", name="oload")
                        nc.sync.dma_start(
                            out=t[:, :],
                            in_=outr[128 * r:128 * (r + 1),
                                     512 * half:512 * (half + 1)])
                        t16 = p10.tile([128, 512], f16, tag="o16", name="o16")
                        nc.vector.tensor_copy(out=t16[:, :], in_=t[:, :])
                        nc.sync.dma_start(
                            out=OUT[128 * r:128 * (r + 1),
                                    512 * half:512 * (half + 1)],
                            in_=t16[:, :])
        kb.ctx.close()
    nc.compile()
    return nc


# ================= runner =================
def make_jit(nc, mesh):
    install_neuronx_cc_hook()
    pname = nc.partition_id_tensor.name if nc.partition_id_tensor else None
    in_names, out_names, out_avals = [], [], []
    for alloc in nc.m.functions[0].allocations:
        if not isinstance(alloc, mybir.MemoryLocationSet):
            continue
        name = alloc.memorylocations[0].name
        if alloc.kind == "ExternalInput":
            if name != pname:
                in_names.append(name)
        elif alloc.kind == "ExternalOutput":
            out_names.append(name)
            out_avals.append(jax.core.ShapedArray(
                tuple(alloc.tensor_shape), mybir.dt.np(alloc.dtype)))
    n_params = len(in_names)
    all_names = list(in_names)
    if pname is not None:
        all_names.append(pname)
    all_names = tuple(all_names)

    def _body(*args):
        operands = list(args)
        if pname is not None:
            operands.append(partition_id_tensor())
        outs = _bass_exec_p.bind(
            *operands, out_avals=tuple(out_avals), in_names=all_names,
            out_names=tuple(out_names), lowering_input_output_aliases=(),
            sim_require_finite=True, sim_require_nnan=True, nc=nc)
        return tuple(outs)

    P = PartitionSpec
    fn = jax.jit(
        shard_map(_body, mesh=mesh,
                  in_specs=(P("core"),) * n_params,
                  out_specs=(P("core"),) * len(out_names), check_rep=False),
        keep_unused=True)
    return fn, in_names, out_names, out_avals


class Chain:
    def __init__(self, ncs):
        self.mesh = Mesh(np.asarray(jax.devices()[:8]), ("core",))
        self.jits = [make_jit(nc, self.mesh) for nc in ncs]

    def run(self, host_inputs):
        """host_inputs: dict name -> np array (8*rows, cols). Returns bufs."""
        bufs = dict(host_inputs)
        for fn, in_names, out_names, out_avals in self.jits:
            args = [bufs[n] for n in in_names]
            outs = fn(*args)
            bufs.update(zip(out_names, outs))
        return bufs


_CHAIN = None


def get_chain():
    global _CHAIN
    if _CHAIN is None:
        ncs = [build_k1(), build_k2(0), build_k2(1), build_k3(0), build_k3(1)]
        _CHAIN = Chain(ncs)
    return _CHAIN


# ================= host packing =================
def pack_inputs(inputs):
    hidden = np.asarray(inputs["hidden_states"], np.float32)
    Wq = np.asarray(inputs["Wq"], np.float32)
    Wk = np.asarray(inputs["Wk"], np.float32)
    Wv = np.asarray(inputs["Wv"], np.float32)
    Wb = np.asarray(inputs["Wb"], np.float32)
    W1 = np.asarray(inputs["gate_W1"], np.float32)
    W2 = np.asarray(inputs["gate_W2"], np.float32)
    b1 = np.asarray(inputs["gate_b1"], np.float32)
    b2 = np.asarray(inputs["gate_b2"], np.float32)
    cpb = np.asarray(inputs["gate_copy_bias"], np.float32)
    ltp = np.asarray(inputs["gate_log_temp"], np.float32)
    Wo = np.asarray(inputs["Wo"], np.float32)
    normw = np.asarray(inputs["o_norm_w"], np.float32)
    cq = np.asarray(inputs["conv_q_w"], np.float32)
    ck = np.asarray(inputs["conv_k_w"], np.float32)
    cv = np.asarray(inputs["conv_v_w"], np.float32)
    firs = np.asarray(inputs["fir_short_filt"], np.float32).reshape(NH * DV, FIRS)
    firl = np.asarray(inputs["fir_long_filt"], np.float32).reshape(NH * DV, FIRL)

    iden = np.eye(128, dtype=np.float32)
    a = np.arange(128)
    negU = np.where(a[:, None] < a[None, :], -1.0, 0.0).astype(np.float32)
    uincl = np.where(a[:, None] <= a[None, :], 1.0, 0.0).astype(np.float32)
    onesm = np.ones((128, 128), np.float32)

    def pk(spec, offs, tot, parts, dt):
        buf = np.empty(tot, dt)
        for nm, shp in spec:
            buf[offs[nm]:offs[nm] + int(np.prod(shp))] = \
                parts[nm].astype(dt).ravel()
        return buf.reshape(1, -1)

    per = {k: [] for k in ("hhalf", "wpk1", "cpk1", "cpk2", "wpk3", "cpk3")}
    for c in range(8):
        b, hl = c // 2, c % 2
        rows = slice(512 * hl, 512 * (hl + 1))
        heads = [2 * hl, 2 * hl + 1]
        invt = np.exp(-ltp[heads])
        b2eff = np.stack([b2 + np.array([0, 0, 0, cpb[hh] * DECAY], np.float32)
                          for hh in heads], 1)
        per["hhalf"].append(np.ascontiguousarray(
            hidden[b, 2048 * hl:2048 * (hl + 1), :].astype(np.float16)))
        per["wpk1"].append(pk(WPK1, W1OFF, W1TOT,
                              {"wqT": Wq[rows].T, "wkT": Wk[rows].T,
                               "wvT": Wv[rows].T, "wbT": Wb[heads].T},
                              np.float16))
        per["cpk1"].append(pk(CPK1, C1OFF, C1TOT,
                              {"convq": cq[rows], "convk": ck[rows],
                               "convv": cv[rows], "firs": firs[rows],
                               "firl": firl[rows]}, np.float32))
        per["cpk2"].append(pk(CPK2, C2OFF, C2TOT,
                              {"iden": iden, "negU": negU, "uincl": uincl,
                               "ones": onesm}, np.float32))
        per["wpk3"].append(pk(WPK3, W3OFF, W3TOT,
                              {"w1hT": W1[:, :HS].T, "woT": Wo[:, rows].T},
                              np.float16))
        per["cpk3"].append(pk(CPK3, C3OFF, C3TOT,
                              {"ones": onesm, "w1sT": W1[:, HS:HS + 16].T,
                               "w2T": W2.T, "b1": b1.reshape(HS, 1),
                               "normw": normw.reshape(DV, 1),
                               "b2s": b2eff * invt[None, :],
                               "invt": np.broadcast_to(invt[None, :], (4, 2))},
                              np.float32))
    return {k: np.concatenate(v, axis=0) for k, v in per.items()}


def unpack_output(out_global):
    o = np.asarray(out_global).reshape(8, 2048, HS)
    out = np.empty((B, L, HS), np.float32)
    for c in range(8):
        b, hl = c // 2, c % 2
        out[b, 2048 * hl:2048 * (hl + 1), :] = o[c]
    return out


def kernel(**inputs):
    chain = get_chain()
    host = pack_inputs(inputs)
    bufs = chain.run(host)
    return unpack_output(bufs["out"])


# ================= warm-load at import =================
def _warmup():
    chain = get_chain()
    host = {
        "hhalf": np.zeros((8 * 2048, HS), np.float16),
        "wpk1": np.zeros((8, W1TOT), np.float16),
        "cpk1": np.zeros((8, C1TOT), np.float32),
        "cpk2": np.zeros((8, C2TOT), np.float32),
        "wpk3": np.zeros((8, W3TOT), np.float16),
        "cpk3": np.zeros((8, C3TOT), np.float32),
    }
    bufs = chain.run(host)
    np.asarray(bufs["out"])
    return chain


try:
    _warmup()
except Exception:
    _CHAIN = None  # fall back to building lazily inside kernel()


# revision 3
# speedup vs baseline: 32.5027x; 1.1054x over previous
"""nn_DeltaNet_31877247271467 — fully-fused TRN2 Bass kernel (8 NeuronCores).

Sharding: core c = (batch c//2, head-half c%2).  hidden/output move host<->device
as fp16; all device compute is fp32 (fp16 operands feed the big matmuls).
The forward pass runs as 5 small chained NEFFs with device-resident
intermediates (AllGather pairs for hidden, group-of-4 AllGather for q/k/v
weights, ReduceScatter pairs for the output projection partials).  NEFFs are
built, compiled, and warm-loaded at import.
"""

import sys

sys.path.insert(0, "/opt/trn_rl_repo")

import numpy as np
from contextlib import ExitStack

import jax
import concourse.bass as bass
import concourse.bacc as bacc
import concourse.tile as tile
from concourse import mybir
from concourse.bass2jax import _bass_exec_p, install_neuronx_cc_hook, partition_id_tensor
from jax.experimental.shard_map import shard_map
from jax.sharding import Mesh, PartitionSpec

f32, f16, bf16 = mybir.dt.float32, mybir.dt.float16, mybir.dt.bfloat16
AF = mybir.ActivationFunctionType
OP = mybir.AluOpType

B, L, HS = 4, 4096, 1024
NH, DK, DV = 4, 256, 256
CONV, FIRS, FIRL = 4, 5, 64
DECAY = 1.0 - 1.0 / 3000.0
EPS_FLOOR = 0.08 * DECAY
RMS_EPS = 1e-05
CH = 512
NB = L // 512
PAIRS = [[0, 1], [2, 3], [4, 5], [6, 7]]

WPK1 = [("wqT", (HS, CH)), ("wkT", (HS, CH)), ("wvT", (HS, CH)), ("wbT", (HS, 2))]
CPK1 = [("convq", (CH, CONV)), ("convk", (CH, CONV)), ("convv", (CH, CONV)),
        ("firs", (CH, FIRS)), ("firl", (CH, FIRL))]
CPK2 = [("iden", (128, 128)), ("negU", (128, 128)), ("uincl", (128, 128)),
        ("ones", (128, 128))]
WPK3 = [("w1hT", (HS, HS)), ("woT", (CH, HS))]
CPK3 = [("ones", (128, 128)), ("w1sT", (16, HS)), ("w2T", (HS, 4)),
        ("b1", (HS, 1)), ("normw", (DV, 1)), ("b2s", (4, 2)), ("invt", (4, 2))]


def _offsets(spec):
    offs, o = {}, 0
    for name, shp in spec:
        offs[name] = o
        o += int(np.prod(shp))
    return offs, o


W1OFF, W1TOT = _offsets(WPK1)
C1OFF, C1TOT = _offsets(CPK1)
C2OFF, C2TOT = _offsets(CPK2)
W3OFF, W3TOT = _offsets(WPK3)
C3OFF, C3TOT = _offsets(CPK3)


def _ap(dram_t, off, ap):
    return bass.AP(tensor=dram_t[0, :].tensor, offset=off, ap=ap)


class KB:
    """Shared per-kernel build helpers."""

    def __init__(self):
        self.nc = bacc.Bacc("TRN2", target_bir_lowering=False, debug=False,
                            num_devices=8)

    def start(self, tc):
        self.tc = tc
        self.ctx = ExitStack()
        self.const = self.ctx.enter_context(tc.tile_pool(name="const", bufs=1))
        self.dram = self.ctx.enter_context(
            tc.tile_pool(name="dram", bufs=1, space="DRAM"))
        self.ps = self.ctx.enter_context(
            tc.tile_pool(name="ps", bufs=7, space="PSUM"))
        self.sc = self.ctx.enter_context(tc.tile_pool(name="sc", bufs=10))

    def psum(self, shape):
        return self.ps.tile(shape, f32, tag="pp", name="pp")

    def scratch(self, shape, dt=f32):
        return self.sc.tile(shape, dt, tag="s512", name="s512")

    def ld2(self, src_dram, name, shape, off, dt=f32, parts=None):
        p = parts or shape[0]
        t = self.const.tile([p, shape[1]], dt, tag="c_" + name, name="c_" + name)
        self.nc.sync.dma_start(out=t[:shape[0], :],
                               in_=_ap(src_dram, off,
                                       [[shape[1], shape[0]], [1, shape[1]]]))
        return t

    def ld3(self, src_dram, name, ktiles, inner, off, dt=f32):
        t = self.const.tile([128, ktiles, inner], dt, tag="c3_" + name,
                            name="c3_" + name)
        self.nc.sync.dma_start(
            out=t[:, :, :],
            in_=_ap(src_dram, off,
                    [[inner, 128], [128 * inner, ktiles], [1, inner]]))
        return t


# ================= K1 =================
def build_k1():
    kb = KB()
    nc = kb.nc
    HH = nc.dram_tensor("hhalf", [L // 2, HS], f16, kind="ExternalInput")
    WP = nc.dram_tensor("wpk1", [1, W1TOT // 4], f16, kind="ExternalInput")
    CP = nc.dram_tensor("cpk1", [1, C1TOT], f32, kind="ExternalInput")
    HID16 = nc.dram_tensor("hid16", [L, HS], f16, kind="ExternalOutput")
    QC = nc.dram_tensor("qc", [CH, L], f32, kind="ExternalOutput")
    KC = nc.dram_tensor("kc", [CH, L], f32, kind="ExternalOutput")
    VC = nc.dram_tensor("vc", [CH, L + 63], f32, kind="ExternalOutput")
    FS = nc.dram_tensor("fs", [CH, L], f32, kind="ExternalOutput")
    FL = nc.dram_tensor("fl", [CH, L], f32, kind="ExternalOutput")
    BETA = nc.dram_tensor("beta", [2, L], f32, kind="ExternalOutput")
    with tile.TileContext(nc) as tc:
        kb.start(tc)
        wpiece = kb.dram.tile([1, W1TOT // 4], f16)
        wfull = kb.dram.tile([1, W1TOT], f16)
        nc.gpsimd.dma_start(wpiece[:, :], WP[:, :])
        nc.gpsimd.collective_compute(
            "AllGather", OP.bypass, replica_groups=[[0, 2, 4, 6], [1, 3, 5, 7]],
            ins=[wpiece.opt()], outs=[wfull.opt()])
        wq = kb.ld3(wfull, "wqT", 8, CH, W1OFF["wqT"], f16)
        wk = kb.ld3(wfull, "wkT", 8, CH, W1OFF["wkT"], f16)
        wv = kb.ld3(wfull, "wvT", 8, CH, W1OFF["wvT"], f16)
        wb = kb.ld3(wfull, "wbT", 8, 2, W1OFF["wbT"], f16)
        conv_t = {n: kb.ld3(CP, "conv" + n, 4, CONV, C1OFF["conv" + n])
                  for n in "qkv"}
        firs_t = kb.ld3(CP, "firs", 4, FIRS, C1OFF["firs"])
        firl_t = kb.ld3(CP, "firl", 4, FIRL, C1OFF["firl"])
        zpad = kb.const.tile([128, 64], f32)
        nc.vector.memset(zpad[:, :], 0.0)

        hid_half = kb.dram.tile([L // 2, HS], f16)
        hid = kb.dram.tile([L, HS], f16)
        raw = {n: kb.dram.tile([CH, L + 3], f32, tag="raw" + n, name="raw" + n)
               for n in "qkv"}
        nc.gpsimd.dma_start(hid_half[:, :], HH[:, :])
        nc.gpsimd.collective_compute(
            "AllGather", OP.bypass, replica_groups=PAIRS,
            ins=[hid_half.opt()], outs=[hid.opt()])
        nc.gpsimd.dma_start(HID16[:, :], hid[:, :])
        hidT = hid[:, :].rearrange("a b -> b a")

        for n in "qkv":
            for m in range(4):
                nc.sync.dma_start(out=raw[n][128 * m:128 * (m + 1), 0:3],
                                  in_=zpad[:, 0:3])
        for m in range(4):
            nc.sync.dma_start(out=VC[128 * m:128 * (m + 1), 0:63],
                              in_=zpad[:, 0:63])

        with tc.tile_pool(name="p1", bufs=2) as p1:
            for tb in range(NB):
                t0 = tb * 512
                ht = []
                for k in range(8):
                    h = p1.tile([128, 512], f16, tag=f"ht{k}", name=f"ht{k}")
                    nc.sync.dma_start(
                        out=h[:, :], in_=hidT[128 * k:128 * (k + 1), t0:t0 + 512])
                    ht.append(h)
                for (wt, dst) in ((wq, raw["q"]), (wk, raw["k"]), (wv, raw["v"])):
                    for m in range(4):
                        p = kb.psum([128, 512])
                        for k in range(8):
                            nc.tensor.matmul(p[:, :],
                                             wt[:, k, 128 * m:128 * (m + 1)],
                                             ht[k][:, :], start=(k == 0),
                                             stop=(k == 7))
                        sb = kb.scratch([128, 512])
                        nc.scalar.copy(out=sb[:, :], in_=p[:, :])
                        nc.sync.dma_start(
                            out=dst[128 * m:128 * (m + 1), 3 + t0:3 + t0 + 512],
                            in_=sb[:, :])
                pb = kb.psum([2, 512])
                for k in range(8):
                    nc.tensor.matmul(pb[:, :], wb[:, k, :], ht[k][:, :],
                                     start=(k == 0), stop=(k == 7))
                bsg = kb.scratch([2, 512])
                nc.scalar.activation(out=bsg[:, :], in_=pb[:, :], func=AF.Sigmoid)
                nc.sync.dma_start(out=BETA[:, t0:t0 + 512], in_=bsg[:, :])

        # conv + silu, full width
        with tc.tile_pool(name="p2", bufs=2) as p2:
            for n, dst, voff in (("q", QC, 0), ("k", KC, 0), ("v", VC, 63)):
                for m in range(4):
                    w = p2.tile([128, L + 3], f32, tag="convw", name="convw")
                    nc.sync.dma_start(out=w[:, :],
                                      in_=raw[n][128 * m:128 * (m + 1), :])
                    acc = p2.tile([128, L], f32, tag="convacc", name="convacc")
                    nc.vector.tensor_scalar(out=acc[:, :], in0=w[:, 0:L],
                                            scalar1=conv_t[n][:, m, 0:1],
                                            scalar2=None, op0=OP.mult)
                    for j in range(1, 4):
                        nc.vector.scalar_tensor_tensor(
                            out=acc[:, :], in0=w[:, j:j + L],
                            scalar=conv_t[n][:, m, j:j + 1], in1=acc[:, :],
                            op0=OP.mult, op1=OP.add)
                    sb = p2.tile([128, L], f32, tag="convout", name="convout")
                    nc.scalar.activation(out=sb[:, :], in_=acc[:, :], func=AF.Silu)
                    nc.sync.dma_start(
                        out=dst[128 * m:128 * (m + 1), voff:voff + L], in_=sb[:, :])
        # FIR, full width
        with tc.tile_pool(name="p3", bufs=2) as p3:
            for m in range(4):
                w = p3.tile([128, L + 63], f32, tag="firw", name="firw")
                nc.sync.dma_start(out=w[:, :], in_=VC[128 * m:128 * (m + 1), :])
                accl = p3.tile([128, L], f32, tag="firaccl", name="firaccl")
                nc.vector.tensor_scalar(out=accl[:, :], in0=w[:, 0:L],
                                        scalar1=firl_t[:, m, 0:1], scalar2=None,
                                        op0=OP.mult)
                for j in range(1, FIRL):
                    nc.vector.scalar_tensor_tensor(
                        out=accl[:, :], in0=w[:, j:j + L],
                        scalar=firl_t[:, m, j:j + 1], in1=accl[:, :],
                        op0=OP.mult, op1=OP.add)
                nc.sync.dma_start(out=FL[128 * m:128 * (m + 1), :], in_=accl[:, :])
                accs = p3.tile([128, L], f32, tag="firaccs", name="firaccs")
                nc.vector.tensor_scalar(out=accs[:, :], in0=w[:, 59:59 + L],
                                        scalar1=firs_t[:, m, 0:1], scalar2=None,
                                        op0=OP.mult)
                for j in range(1, FIRS):
                    nc.vector.scalar_tensor_tensor(
                        out=accs[:, :], in0=w[:, 59 + j:59 + j + L],
                        scalar=firs_t[:, m, j:j + 1], in1=accs[:, :],
                        op0=OP.mult, op1=OP.add)
                nc.sync.dma_start(out=FS[128 * m:128 * (m + 1), :], in_=accs[:, :])
        kb.ctx.close()
    nc.compile()
    return nc


# ================= K2 (per local head) =================
def build_k2(h):
    kb = KB()
    nc = kb.nc
    QC = nc.dram_tensor("qc", [CH, L], f32, kind="ExternalInput")
    KC = nc.dram_tensor("kc", [CH, L], f32, kind="ExternalInput")
    VC = nc.dram_tensor("vc", [CH, L + 63], f32, kind="ExternalInput")
    BETA = nc.dram_tensor("beta", [2, L], f32, kind="ExternalInput")
    CP = nc.dram_tensor("cpk2", [1, C2TOT], f32, kind="ExternalInput")
    ODC = nc.dram_tensor(f"odc{h}", [256, L], f32, kind="ExternalOutput")
    r0 = 256 * h
    with tile.TileContext(nc) as tc:
        kb.start(tc)
        iden = kb.ld2(CP, "iden", (128, 128), C2OFF["iden"])
        negU = kb.ld2(CP, "negU", (128, 128), C2OFF["negU"])
        uincl = kb.ld2(CP, "uincl", (128, 128), C2OFF["uincl"])
        ones = kb.ld2(CP, "ones", (128, 128), C2OFF["ones"])
        epsl2 = kb.const.tile([128, 1], f32)
        nc.vector.memset(epsl2[:, :], 1e-6)

        with tc.tile_pool(name="sp", bufs=1) as sp, \
             tc.tile_pool(name="stash", bufs=3) as stash, \
             tc.tile_pool(name="qkv", bufs=2) as qkv:
            S_sb = [sp.tile([128, 256], f32, tag=f"S{j}", name=f"S{j}")
                    for j in range(2)]
            for j in range(2):
                nc.vector.memset(S_sb[j][:, :], 0.0)
            for blk in range(NB):
                t0 = blk * 512
                qd, kd, vd = [], [], []
                for j in range(2):
                    rj = r0 + 128 * j
                    q_ = qkv.tile([128, 512], f32, tag=f"qd{j}", name=f"qd{j}")
                    nc.sync.dma_start(out=q_[:, :], in_=QC[rj:rj + 128, t0:t0 + 512])
                    qd.append(q_)
                    k_ = qkv.tile([128, 512], f32, tag=f"kd{j}", name=f"kd{j}")
                    nc.sync.dma_start(out=k_[:, :], in_=KC[rj:rj + 128, t0:t0 + 512])
                    kd.append(k_)
                    v_ = qkv.tile([128, 512], f32, tag=f"vd{j}", name=f"vd{j}")
                    nc.sync.dma_start(out=v_[:, :],
                                      in_=VC[rj:rj + 128, 63 + t0:63 + t0 + 512])
                    vd.append(v_)
                bb = qkv.tile([1, 512], f32, tag="bb", name="bb")
                nc.sync.dma_start(out=bb[:, :], in_=BETA[h:h + 1, t0:t0 + 512])
                rq_b = qkv.tile([1, 512], f32, tag="rqb", name="rqb")
                rk_b = qkv.tile([1, 512], f32, tag="rkb", name="rkb")
                for (dsrc, rdst) in ((qd, rq_b), (kd, rk_b)):
                    sqt = [kb.scratch([128, 512]) for _ in range(2)]
                    for j in range(2):
                        nc.scalar.activation(out=sqt[j][:, :], in_=dsrc[j][:, :],
                                             func=AF.Square)
                    pssum = kb.psum([1, 512])
                    for j in range(2):
                        nc.tensor.matmul(pssum[:, :], ones[:, 0:1], sqt[j][:, :],
                                         start=(j == 0), stop=(j == 1))
                    nrm = kb.scratch([1, 512])
                    nc.scalar.activation(out=nrm[:, :], in_=pssum[:, :],
                                         func=AF.Sqrt, bias=epsl2[0:1, :])
                    nc.vector.reciprocal(out=rdst[:, :], in_=nrm[:, :])
                for cc in range(4):
                    c0 = t0 + cc * 128
                    s0 = cc * 128
                    pcol = kb.psum([128, 3])
                    nc.tensor.matmul(pcol[:, 0:1], bb[0:1, s0:s0 + 128],
                                     ones[0:1, 0:1], start=True, stop=True)
                    nc.tensor.matmul(pcol[:, 1:2], rq_b[0:1, s0:s0 + 128],
                                     ones[0:1, 0:1], start=True, stop=True)
                    nc.tensor.matmul(pcol[:, 2:3], rk_b[0:1, s0:s0 + 128],
                                     ones[0:1, 0:1], start=True, stop=True)
                    cols = stash.tile([128, 3], f32, tag="cols", name="cols")
                    nc.vector.tensor_copy(out=cols[:, :], in_=pcol[:, :])
                    bcol, rqcol, rkcol = cols[:, 0:1], cols[:, 1:2], cols[:, 2:3]
                    brk = kb.scratch([1, 128])
                    nc.vector.tensor_tensor(out=brk[:, :],
                                            in0=bb[0:1, s0:s0 + 128],
                                            in1=rk_b[0:1, s0:s0 + 128], op=OP.mult)
                    prb = kb.psum([128, 128])
                    nc.tensor.matmul(prb[:, :], ones[0:1, :], brk[:, :],
                                     start=True, stop=True)
                    pg = kb.psum([128, 128])
                    for j in range(2):
                        nc.tensor.matmul(pg[:, :], kd[j][:, s0:s0 + 128],
                                         kd[j][:, s0:s0 + 128], start=(j == 0),
                                         stop=(j == 1))
                    ptmp = kb.scratch([128, 128])
                    nc.vector.scalar_tensor_tensor(out=ptmp[:, :], in0=pg[:, :],
                                                   scalar=rkcol, in1=negU[:, :],
                                                   op0=OP.mult, op1=OP.mult)
                    P_sb = kb.scratch([128, 128])
                    nc.vector.tensor_tensor(out=P_sb[:, :], in0=ptmp[:, :],
                                            in1=prb[:, :], op=OP.mult)
                    pa = kb.psum([128, 128])
                    for j in range(2):
                        nc.tensor.matmul(pa[:, :], kd[j][:, s0:s0 + 128],
                                         qd[j][:, s0:s0 + 128], start=(j == 0),
                                         stop=(j == 1))
                    attnT = stash.tile([128, 128], f32, tag="attnT", name="attnT")
                    nc.vector.scalar_tensor_tensor(out=attnT[:, :], in0=pa[:, :],
                                                   scalar=rkcol, in1=uincl[:, :],
                                                   op0=OP.mult, op1=OP.mult)
                    pkt = kb.psum([128, 256])
                    pvt = kb.psum([128, 256])
                    for j in range(2):
                        nc.tensor.matmul(pkt[:, 128 * j:128 * (j + 1)],
                                         kd[j][:, s0:s0 + 128], iden[:, :],
                                         start=True, stop=True)
                        nc.tensor.matmul(pvt[:, 128 * j:128 * (j + 1)],
                                         vd[j][:, s0:s0 + 128], iden[:, :],
                                         start=True, stop=True)
                    kntm = stash.tile([128, 256], f32, tag="kntm", name="kntm")
                    nc.vector.tensor_scalar(out=kntm[:, :], in0=pkt[:, :],
                                            scalar1=rkcol, scalar2=None,
                                            op0=OP.mult)
                    uw = stash.tile([128, 512], f32, tag="uw", name="uw")
                    nc.vector.tensor_scalar(out=uw[:, 0:256], in0=pvt[:, :],
                                            scalar1=bcol, scalar2=None,
                                            op0=OP.mult)
                    nc.vector.tensor_scalar(out=uw[:, 256:512], in0=kntm[:, :],
                                            scalar1=bcol, scalar2=None,
                                            op0=OP.mult)
                    for lvl in range(7):
                        puw = kb.psum([128, 512])
                        nc.tensor.matmul(puw[:, :], P_sb[:, :], uw[:, :],
                                         start=True, stop=True)
                        nc.vector.tensor_tensor(out=uw[:, :], in0=puw[:, :],
                                                in1=uw[:, :], op=OP.add)
                        if lvl < 6:
                            ptr = kb.psum([128, 128])
                            nc.tensor.matmul(ptr[:, :], P_sb[:, :], iden[:, :],
                                             start=True, stop=True)
                            PT_sb = kb.scratch([128, 128])
                            nc.vector.tensor_copy(out=PT_sb[:, :], in_=ptr[:, :])
                            psq = kb.psum([128, 128])
                            nc.tensor.matmul(psq[:, :], PT_sb[:, :], P_sb[:, :],
                                             start=True, stop=True)
                            P_sb = kb.scratch([128, 128])
                            nc.vector.tensor_copy(out=P_sb[:, :], in_=psq[:, :])
                    pwt = kb.psum([128, 256])
                    for j in range(2):
                        nc.tensor.matmul(pwt[:, 128 * j:128 * (j + 1)],
                                         uw[:, 256 + 128 * j:256 + 128 * (j + 1)],
                                         iden[:, :], start=True, stop=True)
                    wtcm = stash.tile([128, 256], f32, tag="wtcm", name="wtcm")
                    nc.vector.tensor_copy(out=wtcm[:, :], in_=pwt[:, :])

                    pws = kb.psum([128, 256])
                    for j in range(2):
                        nc.tensor.matmul(pws[:, :], wtcm[:, 128 * j:128 * (j + 1)],
                                         S_sb[j][:, :], start=(j == 0),
                                         stop=(j == 1))
                    ui = stash.tile([128, 256], f32, tag="ui", name="ui")
                    nc.vector.tensor_tensor(out=ui[:, :], in0=uw[:, 0:256],
                                            in1=pws[:, :], op=OP.subtract)
                    po = kb.psum([128, 256])
                    for j in range(2):
                        nc.tensor.matmul(po[:, :], qd[j][:, s0:s0 + 128],
                                         S_sb[j][:, :], start=(j == 0), stop=False)
                    nc.tensor.matmul(po[:, :], attnT[:, :], ui[:, :],
                                     start=False, stop=True)
                    otm = kb.scratch([128, 256])
                    nc.vector.tensor_scalar(out=otm[:, :], in0=po[:, :],
                                            scalar1=rqcol, scalar2=None,
                                            op0=OP.mult)
                    for j in range(2):
                        pds = kb.psum([128, 256])
                        nc.tensor.matmul(pds[:, :], kntm[:, 128 * j:128 * (j + 1)],
                                         ui[:, :], start=True, stop=True)
                        nc.vector.tensor_tensor(out=S_sb[j][:, :],
                                                in0=S_sb[j][:, :],
                                                in1=pds[:, :], op=OP.add)
                    pot = kb.psum([128, 256])
                    for j in range(2):
                        nc.tensor.matmul(pot[:, 128 * j:128 * (j + 1)],
                                         otm[:, 128 * j:128 * (j + 1)],
                                         iden[:, :], start=True, stop=True)
                    osb = kb.scratch([128, 256])
                    nc.scalar.copy(out=osb[:, :], in_=pot[:, :])
                    for j in range(2):
                        nc.sync.dma_start(
                            out=ODC[128 * j:128 * (j + 1), c0:c0 + 128],
                            in_=osb[:, 128 * j:128 * (j + 1)])
        kb.ctx.close()
    nc.compile()
    return nc


# ================= K3 (per time half) =================
def build_k3(p):
    kb = KB()
    nc = kb.nc
    HID16 = nc.dram_tensor("hid16", [L, HS], f16, kind="ExternalInput")
    VC = nc.dram_tensor("vc", [CH, L + 63], f32, kind="ExternalInput")
    FS = nc.dram_tensor("fs", [CH, L], f32, kind="ExternalInput")
    FL = nc.dram_tensor("fl", [CH, L], f32, kind="ExternalInput")
    OD0 = nc.dram_tensor("odc0", [256, L], f32, kind="ExternalInput")
    OD1 = nc.dram_tensor("odc1", [256, L], f32, kind="ExternalInput")
    WP = nc.dram_tensor("wpk3", [1, W3TOT], f16, kind="ExternalInput")
    CP = nc.dram_tensor("cpk3", [1, C3TOT], f32, kind="ExternalInput")
    if p == 0:
        OUTP = nc.dram_tensor("outp0", [L // 2, HS], f32, kind="ExternalOutput")
    else:
        O0 = nc.dram_tensor("outp0", [L // 2, HS], f32, kind="ExternalInput")
        OUT = nc.dram_tensor("out", [L // 2, HS], f16, kind="ExternalOutput")
    with tile.TileContext(nc) as tc:
        kb.start(tc)
        if p == 1:
            outp_full = kb.dram.tile([L, HS], f32)
            outr = kb.dram.tile([L // 2, HS], f32)
            OUTP = outp_full[L // 2:L, :]
            nc.gpsimd.dma_start(outp_full[0:L // 2, :], O0[:, :])
        nc_ = nc
        ones = kb.ld2(CP, "ones", (128, 128), C3OFF["ones"])
        w1s = kb.ld2(CP, "w1sT", (16, HS), C3OFF["w1sT"], parts=32)
        b2s_t = kb.ld2(CP, "b2s", (4, 2), C3OFF["b2s"], parts=32)
        invt_t = kb.ld2(CP, "invt", (4, 2), C3OFF["invt"], parts=32)
        w2f = kb.ld3(CP, "w2T", 8, 4, C3OFF["w2T"])
        w2_t = kb.const.tile([128, 8, 4], bf16)
        nc.vector.tensor_copy(out=w2_t[:, :, :], in_=w2f[:, :, :])
        b1_t = kb.const.tile([128, 8], f32)
        nc.sync.dma_start(out=b1_t[:, :],
                          in_=_ap(CP, C3OFF["b1"], [[1, 128], [128, 8]]))
        normw_t = kb.const.tile([128, 2], f32)
        nc.sync.dma_start(out=normw_t[:, :],
                          in_=_ap(CP, C3OFF["normw"], [[1, 128], [128, 2]]))
        epsrms = kb.const.tile([128, 1], f32)
        nc.vector.memset(epsrms[:, :], RMS_EPS)
        w1 = kb.ld3(WP, "w1hT", 8, HS, W3OFF["w1hT"], f16)
        wo16 = kb.ld3(WP, "woT", 4, HS, W3OFF["woT"], f16)
        hidT = HID16[:, :].rearrange("a b -> b a")

        STATS_SRC = [("fs", FS, 0), ("fl", FL, 0), ("od", None, 0), ("v", VC, 63)]
        with tc.tile_pool(name="p7", bufs=2) as p7, \
             tc.tile_pool(name="p7one", bufs=1) as p7one, \
             tc.tile_pool(name="onp", bufs=2) as onp:
            for tbl in range(4):
                tb = 4 * p + tbl
                t0 = tb * 512
                tl0 = tbl * 512
                ht = []
                for k in range(8):
                    hh = p7.tile([128, 512], f16, tag=f"ht{k}", name=f"ht{k}")
                    nc.sync.dma_start(
                        out=hh[:, :], in_=hidT[128 * k:128 * (k + 1), t0:t0 + 512])
                    ht.append(hh)
                on_all = []
                for h in range(2):
                    r0 = 256 * h
                    odh = OD0 if h == 0 else OD1

                    def src_rows(ti, j):
                        nm, src, voff = STATS_SRC[ti]
                        if nm == "od":
                            return odh[128 * j:128 * (j + 1), t0:t0 + 512]
                        return src[r0 + 128 * j:r0 + 128 * (j + 1),
                                   voff + t0:voff + t0 + 512]

                    stats_blk = p7.tile([32, 512], f32, tag="statsblk",
                                        name="statsblk")
                    for ti in range(4):
                        bt = [p7.tile([128, 512], f32, tag=f"bt{j}", name=f"bt{j}")
                              for j in range(2)]
                        for j in range(2):
                            nc.sync.dma_start(out=bt[j][:, :], in_=src_rows(ti, j))
                        sq = [kb.scratch([128, 512]) for _ in range(2)]
                        ab = [kb.scratch([128, 512]) for _ in range(2)]
                        for j in range(2):
                            nc.scalar.activation(out=sq[j][:, :], in_=bt[j][:, :],
                                                 func=AF.Square)
                            nc.scalar.activation(out=ab[j][:, :], in_=bt[j][:, :],
                                                 func=AF.Abs)
                        psx = kb.psum([1, 512])
                        psq2 = kb.psum([1, 512])
                        psa = kb.psum([1, 512])
                        for j in range(2):
                            nc.tensor.matmul(psx[:, :], ones[:, 0:1], bt[j][:, :],
                                             start=(j == 0), stop=(j == 1))
                            nc.tensor.matmul(psq2[:, :], ones[:, 0:1], sq[j][:, :],
                                             start=(j == 0), stop=(j == 1))
                            nc.tensor.matmul(psa[:, :], ones[:, 0:1], ab[j][:, :],
                                             start=(j == 0), stop=(j == 1))
                        r = 4 * ti
                        fin = kb.scratch([1, 4, 512])
                        nc.scalar.activation(out=fin[:, 0, :], in_=psx[:, :],
                                             func=AF.Copy, scale=1.0 / 256.0)
                        m2 = kb.scratch([1, 512])
                        nc.vector.tensor_tensor(out=m2[:, :], in0=fin[:, 0, :],
                                                in1=fin[:, 0, :], op=OP.mult)
                        nc.vector.scalar_tensor_tensor(
                            out=fin[:, 1, :], in0=psq2[:, :], scalar=1.0 / 256.0,
                            in1=m2[:, :], op0=OP.mult, op1=OP.subtract)
                        nc.scalar.activation(out=fin[:, 2, :], in_=psa[:, :],
                                             func=AF.Copy, scale=1.0 / 256.0)
                        nc.scalar.activation(out=fin[:, 3, :], in_=psq2[:, :],
                                             func=AF.Sqrt)
                        nc.sync.dma_start(
                            out=stats_blk[r:r + 4, :],
                            in_=bass.AP(tensor=fin.tensor, offset=fin.offset,
                                        ap=[[512, 4], [1, 512]]))
                    # gate MLP
                    h1 = p7one.tile([128, 8, 512], bf16, tag="h1", name="h1")
                    for gm in range(8):
                        pg1 = kb.psum([128, 512])
                        for k in range(8):
                            nc.tensor.matmul(pg1[:, :],
                                             w1[:, k, 128 * gm:128 * (gm + 1)],
                                             ht[k][:, :], start=(k == 0),
                                             stop=False)
                        nc.tensor.matmul(pg1[:, :],
                                         w1s[0:16, 128 * gm:128 * (gm + 1)],
                                         stats_blk[0:16, :], start=False, stop=True)
                        nc.scalar.activation(out=h1[:, gm, :], in_=pg1[:, :],
                                             func=AF.Gelu_apprx_tanh,
                                             bias=b1_t[:, gm:gm + 1])
                    pl = kb.psum([4, 512])
                    for k in range(8):
                        nc.tensor.matmul(pl[:, :], w2_t[:, k, :], h1[:, k, :],
                                         start=(k == 0), stop=(k == 7))
                    el = kb.scratch([4, 512])
                    nc.scalar.activation(out=el[:, :], in_=pl[:, :], func=AF.Exp,
                                         scale=invt_t[0:4, h:h + 1],
                                         bias=b2s_t[0:4, h:h + 1])
                    pse = kb.psum([1, 512])
                    nc.tensor.matmul(pse[:, :], ones[0:4, 0:1], el[:, :],
                                     start=True, stop=True)
                    rec = kb.scratch([1, 512])
                    nc.vector.reciprocal(out=rec[:, :], in_=pse[:, :])
                    prr = kb.psum([4, 512])
                    nc.tensor.matmul(prr[:, :], ones[0:1, 0:4], rec[:, :],
                                     start=True, stop=True)
                    wgt = p7.tile([4, 512], f32, tag="wgtt", name="wgtt")
                    nc.vector.tensor_tensor(out=wgt[:, :], in0=el[:, :],
                                            in1=prr[:, :], op=OP.mult)
                    nc.vector.tensor_scalar(out=wgt[:, :], in0=wgt[:, :],
                                            scalar1=1.0 - 4.0 * EPS_FLOOR,
                                            scalar2=EPS_FLOOR, op0=OP.mult,
                                            op1=OP.add)
                    wbc = p7one.tile([128, 4, 512], f32, tag="wbc", name="wbc")
                    for cls in range(4):
                        wrow = kb.scratch([1, 512])
                        nc.sync.dma_start(out=wrow[:, :], in_=wgt[cls:cls + 1, :])
                        pwb = kb.psum([128, 512])
                        nc.tensor.matmul(pwb[:, :], ones[0:1, :], wrow[:, :],
                                         start=True, stop=True)
                        nc.scalar.copy(out=wbc[:, cls, :], in_=pwb[:, :])
                    ob = [p7one.tile([128, 512], f32, tag=f"ob{j}", name=f"ob{j}")
                          for j in range(2)]
                    osq = [kb.scratch([128, 512]) for _ in range(2)]
                    for cls in range(4):
                        for j in range(2):
                            bl = p7.tile([128, 512], f32, tag=f"bl{j}",
                                         name=f"bl{j}")
                            nc.sync.dma_start(out=bl[:, :], in_=src_rows(cls, j))
                            if cls == 0:
                                nc.vector.tensor_tensor(out=ob[j][:, :],
                                                        in0=bl[:, :],
                                                        in1=wbc[:, 0, :],
                                                        op=OP.mult)
                            else:
                                tbr = kb.scratch([128, 512])
                                nc.vector.tensor_tensor(out=tbr[:, :],
                                                        in0=bl[:, :],
                                                        in1=wbc[:, cls, :],
                                                        op=OP.mult)
                                nc.vector.tensor_tensor(out=ob[j][:, :],
                                                        in0=ob[j][:, :],
                                                        in1=tbr[:, :], op=OP.add)
                    for j in range(2):
                        nc.scalar.activation(out=osq[j][:, :], in_=ob[j][:, :],
                                             func=AF.Square)
                    pso = kb.psum([1, 512])
                    for j in range(2):
                        nc.tensor.matmul(pso[:, :], ones[:, 0:1], osq[j][:, :],
                                         start=(j == 0), stop=(j == 1))
                    srt = kb.scratch([1, 512])
                    nc.scalar.activation(out=srt[:, :], in_=pso[:, :],
                                         func=AF.Sqrt, scale=1.0 / 256.0,
                                         bias=epsrms[0:1, :])
                    rre = kb.scratch([1, 512])
                    nc.vector.reciprocal(out=rre[:, :], in_=srt[:, :])
                    prn = kb.psum([128, 512])
                    nc.tensor.matmul(prn[:, :], ones[0:1, :], rre[:, :],
                                     start=True, stop=True)
                    ons = []
                    for j in range(2):
                        on = onp.tile([128, 512], f16, tag=f"on{h}{j}",
                                      name=f"on{h}{j}")
                        nc.vector.scalar_tensor_tensor(
                            out=on[:, :], in0=ob[j][:, :],
                            scalar=normw_t[:, j:j + 1], in1=prn[:, :],
                            op0=OP.mult, op1=OP.mult)
                        ons.append(on)
                    on_all.append(ons)
                for m in range(8):
                    pw = kb.psum([128, 512])
                    for d in range(4):
                        nc.tensor.matmul(pw[:, :],
                                         wo16[:, d, 128 * m:128 * (m + 1)],
                                         on_all[d // 2][d % 2][:, :],
                                         start=(d == 0), stop=(d == 3))
                    owr = kb.scratch([128, 512])
                    nc.vector.tensor_copy(out=owr[:, :], in_=pw[:, :])
                    nc.sync.dma_start(
                        out=OUTP[tl0:tl0 + 512,
                                 128 * m:128 * (m + 1)].rearrange("a b -> b a"),
                        in_=owr[:, :])
        if p == 1:
            nc.gpsimd.collective_compute(
                "ReduceScatter", OP.add, replica_groups=PAIRS,
                ins=[outp_full.opt()], outs=[outr.opt()])
            with tc.tile_pool(name="p10", bufs=2) as p10:
                for r in range(16):
                    for half in range(2):
                        t = p10.tile([128, 512], f32, tag="oload", name="oload")
                        nc.sync.dma_start(
                            out=t[:, :],
                            in_=outr[128 * r:128 * (r + 1),
                                     512 * half:512 * (half + 1)])
                        t16 = p10.tile([128, 512], f16, tag="o16", name="o16")
                        nc.vector.tensor_copy(out=t16[:, :], in_=t[:, :])
                        nc.sync.dma_start(
                            out=OUT[128 * r:128 * (r + 1),
                                    512 * half:512 * (half + 1)],
                            in_=t16[:, :])
        kb.ctx.close()
    nc.compile()
    return nc


# ================= runner =================
def make_jit(nc, mesh):
    install_neuronx_cc_hook()
    pname = nc.partition_id_tensor.name if nc.partition_id_tensor else None
    in_names, out_names, out_avals = [], [], []
    for alloc in nc.m.functions[0].allocations:
        if not isinstance(alloc, mybir.MemoryLocationSet):
            continue
        name = alloc.memorylocations[0].name
        if alloc.kind == "ExternalInput":
            if name != pname:
                in_names.append(name)
        elif alloc.kind == "ExternalOutput":
            out_names.append(name)
            out_avals.append(jax.core.ShapedArray(
                tuple(alloc.tensor_shape), mybir.dt.np(alloc.dtype)))
    n_params = len(in_names)
    all_names = list(in_names)
    if pname is not None:
        all_names.append(pname)
    all_names = tuple(all_names)

    def _body(*args):
        operands = list(args)
        if pname is not None:
            operands.append(partition_id_tensor())
        outs = _bass_exec_p.bind(
            *operands, out_avals=tuple(out_avals), in_names=all_names,
            out_names=tuple(out_names), lowering_input_output_aliases=(),
            sim_require_finite=True, sim_require_nnan=True, nc=nc)
        return tuple(outs)

    P = PartitionSpec
    fn = jax.jit(
        shard_map(_body, mesh=mesh,
                  in_specs=(P("core"),) * n_params,
                  out_specs=(P("core"),) * len(out_names), check_rep=False),
        keep_unused=True)
    return fn, in_names, out_names, out_avals


class Chain:
    def __init__(self, ncs):
        self.mesh = Mesh(np.asarray(jax.devices()[:8]), ("core",))
        self.jits = [make_jit(nc, self.mesh) for nc in ncs]

    def run(self, host_inputs):
        """host_inputs: dict name -> np array (8*rows, cols). Returns bufs."""
        bufs = dict(host_inputs)
        for fn, in_names, out_names, out_avals in self.jits:
            args = [bufs[n] for n in in_names]
            outs = fn(*args)
            bufs.update(zip(out_names, outs))
        return bufs


_CHAIN = None


def get_chain():
    global _CHAIN
    if _CHAIN is None:
        ncs = [build_k1(), build_k2(0), build_k2(1), build_k3(0), build_k3(1)]
        _CHAIN = Chain(ncs)
    return _CHAIN


# ================= host packing =================
def pack_inputs(inputs):
    hidden = np.asarray(inputs["hidden_states"], np.float32)
    Wq = np.asarray(inputs["Wq"], np.float32)
    Wk = np.asarray(inputs["Wk"], np.float32)
    Wv = np.asarray(inputs["Wv"], np.float32)
    Wb = np.asarray(inputs["Wb"], np.float32)
    W1 = np.asarray(inputs["gate_W1"], np.float32)
    W2 = np.asarray(inputs["gate_W2"], np.float32)
    b1 = np.asarray(inputs["gate_b1"], np.float32)
    b2 = np.asarray(inputs["gate_b2"], np.float32)
    cpb = np.asarray(inputs["gate_copy_bias"], np.float32)
    ltp = np.asarray(inputs["gate_log_temp"], np.float32)
    Wo = np.asarray(inputs["Wo"], np.float32)
    normw = np.asarray(inputs["o_norm_w"], np.float32)
    cq = np.asarray(inputs["conv_q_w"], np.float32)
    ck = np.asarray(inputs["conv_k_w"], np.float32)
    cv = np.asarray(inputs["conv_v_w"], np.float32)
    firs = np.asarray(inputs["fir_short_filt"], np.float32).reshape(NH * DV, FIRS)
    firl = np.asarray(inputs["fir_long_filt"], np.float32).reshape(NH * DV, FIRL)

    iden = np.eye(128, dtype=np.float32)
    a = np.arange(128)
    negU = np.where(a[:, None] < a[None, :], -1.0, 0.0).astype(np.float32)
    uincl = np.where(a[:, None] <= a[None, :], 1.0, 0.0).astype(np.float32)
    onesm = np.ones((128, 128), np.float32)

    def pk(spec, offs, tot, parts, dt):
        buf = np.empty(tot, dt)
        for nm, shp in spec:
            buf[offs[nm]:offs[nm] + int(np.prod(shp))] = \
                parts[nm].astype(dt).ravel()
        return buf.reshape(1, -1)

    per = {k: [] for k in ("hhalf", "wpk1", "cpk1", "cpk2", "wpk3", "cpk3")}
    for c in range(8):
        b, hl = c // 2, c % 2
        rows = slice(512 * hl, 512 * (hl + 1))
        heads = [2 * hl, 2 * hl + 1]
        invt = np.exp(-ltp[heads])
        b2eff = np.stack([b2 + np.array([0, 0, 0, cpb[hh] * DECAY], np.float32)
                          for hh in heads], 1)
        per["hhalf"].append(np.ascontiguousarray(
            hidden[b, 2048 * hl:2048 * (hl + 1), :].astype(np.float16)))
        w1flat = pk(WPK1, W1OFF, W1TOT,
                    {"wqT": Wq[rows].T, "wkT": Wk[rows].T,
                     "wvT": Wv[rows].T, "wbT": Wb[heads].T}, np.float16)
        qtr = W1TOT // 4
        per["wpk1"].append(w1flat[:, (c // 2) * qtr:(c // 2 + 1) * qtr])
        per["cpk1"].append(pk(CPK1, C1OFF, C1TOT,
                              {"convq": cq[rows], "convk": ck[rows],
                               "convv": cv[rows], "firs": firs[rows],
                               "firl": firl[rows]}, np.float32))
        per["cpk2"].append(pk(CPK2, C2OFF, C2TOT,
                              {"iden": iden, "negU": negU, "uincl": uincl,
                               "ones": onesm}, np.float32))
        per["wpk3"].append(pk(WPK3, W3OFF, W3TOT,
                              {"w1hT": W1[:, :HS].T, "woT": Wo[:, rows].T},
                              np.float16))
        per["cpk3"].append(pk(CPK3, C3OFF, C3TOT,
                              {"ones": onesm, "w1sT": W1[:, HS:HS + 16].T,
                               "w2T": W2.T, "b1": b1.reshape(HS, 1),
                               "normw": normw.reshape(DV, 1),
                               "b2s": b2eff * invt[None, :],
                               "invt": np.broadcast_to(invt[None, :], (4, 2))},
                              np.float32))
    return {k: np.concatenate(v, axis=0) for k, v in per.items()}


def unpack_output(out_global):
    o = np.asarray(out_global).reshape(8, 2048, HS)
    out = np.empty((B, L, HS), np.float32)
    for c in range(8):
        b, hl = c // 2, c % 2
        out[b, 2048 * hl:2048 * (hl + 1), :] = o[c]
    return out


def kernel(**inputs):
    chain = get_chain()
    host = pack_inputs(inputs)
    bufs = chain.run(host)
    return unpack_output(bufs["out"])


# ================= warm-load at import =================
def _warmup():
    chain = get_chain()
    host = {
        "hhalf": np.zeros((8 * 2048, HS), np.float16),
        "wpk1": np.zeros((8, W1TOT // 4), np.float16),
        "cpk1": np.zeros((8, C1TOT), np.float32),
        "cpk2": np.zeros((8, C2TOT), np.float32),
        "wpk3": np.zeros((8, W3TOT), np.float16),
        "cpk3": np.zeros((8, C3TOT), np.float32),
    }
    bufs = chain.run(host)
    np.asarray(bufs["out"])
    return chain


try:
    _warmup()
except Exception:
    _CHAIN = None  # fall back to building lazily inside kernel()


# revision 4
# speedup vs baseline: 46.0349x; 1.4163x over previous
"""nn_DeltaNet_31877247271467 — fully-fused TRN2 Bass kernel (8 NeuronCores).

Sharding: core c = (batch c//2, head-half c%2).  hidden/output move host<->device
as fp16; all device compute is fp32 (fp16 operands feed the big matmuls).
The forward pass runs as 5 small chained NEFFs with device-resident
intermediates.  Collectives: pair AllGather for hidden, group-of-4 AllGathers
(in K1) deduplicating both weight packs, pair ReduceScatter for the output
projection partials.  NEFFs are built, compiled, and warm-loaded at import.
"""

import sys

sys.path.insert(0, "/opt/trn_rl_repo")

import numpy as np
from contextlib import ExitStack

import jax
import concourse.bass as bass
import concourse.bacc as bacc
import concourse.tile as tile
from concourse import mybir
from concourse.bass2jax import _bass_exec_p, install_neuronx_cc_hook, partition_id_tensor
from jax.experimental.shard_map import shard_map
from jax.sharding import Mesh, PartitionSpec

f32, f16, bf16 = mybir.dt.float32, mybir.dt.float16, mybir.dt.bfloat16
AF = mybir.ActivationFunctionType
OP = mybir.AluOpType

B, L, HS = 4, 4096, 1024
NH, DK, DV = 4, 256, 256
CONV, FIRS, FIRL = 4, 5, 64
DECAY = 1.0 - 1.0 / 3000.0
EPS_FLOOR = 0.08 * DECAY
RMS_EPS = 1e-05
CH = 512
NB = L // 512
PAIRS = [[0, 1], [2, 3], [4, 5], [6, 7]]

WPK1 = [("wqT", (HS, CH)), ("wkT", (HS, CH)), ("wvT", (HS, CH)), ("wbT", (HS, 2))]
CPK1 = [("convq", (CH, CONV)), ("convk", (CH, CONV)), ("convv", (CH, CONV)),
        ("firs", (CH, FIRS)), ("firl", (CH, FIRL))]
CPK2 = [("iden", (128, 128)), ("negU", (128, 128)), ("uincl", (128, 128)),
        ("ones", (128, 128))]
WPK3 = [("w1hT", (HS, HS)), ("woT", (CH, HS))]
CPK3 = [("ones", (128, 128)), ("w1sT", (16, HS)), ("w2T", (HS, 4)),
        ("b1", (HS, 1)), ("normw", (DV, 1)), ("b2s", (4, 2)), ("invt", (4, 2))]


def _offsets(spec):
    offs, o = {}, 0
    for name, shp in spec:
        offs[name] = o
        o += int(np.prod(shp))
    return offs, o


W1OFF, W1TOT = _offsets(WPK1)
C1OFF, C1TOT = _offsets(CPK1)
C2OFF, C2TOT = _offsets(CPK2)
W3OFF, W3TOT = _offsets(WPK3)
C3OFF, C3TOT = _offsets(CPK3)


def _ap(dram_t, off, ap):
    return bass.AP(tensor=dram_t[0, :].tensor, offset=off, ap=ap)


class KB:
    """Shared per-kernel build helpers."""

    def __init__(self):
        self.nc = bacc.Bacc("TRN2", target_bir_lowering=False, debug=False,
                            num_devices=8)

    def start(self, tc):
        self.tc = tc
        self.ctx = ExitStack()
        self.const = self.ctx.enter_context(tc.tile_pool(name="const", bufs=1))
        self.dram = self.ctx.enter_context(
            tc.tile_pool(name="dram", bufs=1, space="DRAM"))
        self.ps = self.ctx.enter_context(
            tc.tile_pool(name="ps", bufs=7, space="PSUM"))
        self.sc = self.ctx.enter_context(tc.tile_pool(name="sc", bufs=10))

    def psum(self, shape):
        return self.ps.tile(shape, f32, tag="pp", name="pp")

    def scratch(self, shape, dt=f32):
        return self.sc.tile(shape, dt, tag="s512", name="s512")

    def ld2(self, src_dram, name, shape, off, dt=f32, parts=None):
        p = parts or shape[0]
        t = self.const.tile([p, shape[1]], dt, tag="c_" + name, name="c_" + name)
        self.nc.sync.dma_start(out=t[:shape[0], :],
                               in_=_ap(src_dram, off,
                                       [[shape[1], shape[0]], [1, shape[1]]]))
        return t

    def ld3(self, src_dram, name, ktiles, inner, off, dt=f32):
        t = self.const.tile([128, ktiles, inner], dt, tag="c3_" + name,
                            name="c3_" + name)
        self.nc.sync.dma_start(
            out=t[:, :, :],
            in_=_ap(src_dram, off,
                    [[inner, 128], [128 * inner, ktiles], [1, inner]]))
        return t


# ================= K1 =================
def build_k1():
    kb = KB()
    nc = kb.nc
    HH = nc.dram_tensor("hhalf", [L // 2, HS], f16, kind="ExternalInput")
    WP = nc.dram_tensor("wpk1", [1, W1TOT // 4], f16, kind="ExternalInput")
    CP = nc.dram_tensor("cpk1", [1, C1TOT], f32, kind="ExternalInput")
    WP3Q = nc.dram_tensor("wpk3q", [1, W3TOT // 4], f16, kind="ExternalInput")
    WP3F = nc.dram_tensor("wpk3f", [1, W3TOT], f16, kind="ExternalOutput")
    HID16 = nc.dram_tensor("hid16", [L, HS], f16, kind="ExternalOutput")
    QC = nc.dram_tensor("qc", [CH, L], f32, kind="ExternalOutput")
    KC = nc.dram_tensor("kc", [CH, L], f32, kind="ExternalOutput")
    VC = nc.dram_tensor("vc", [CH, L + 63], f32, kind="ExternalOutput")
    FS = nc.dram_tensor("fs", [CH, L], f32, kind="ExternalOutput")
    FL = nc.dram_tensor("fl", [CH, L], f32, kind="ExternalOutput")
    BETA = nc.dram_tensor("beta", [2, L], f32, kind="ExternalOutput")
    with tile.TileContext(nc) as tc:
        kb.start(tc)
        wpiece = kb.dram.tile([1, W1TOT // 4], f16)
        wfull = kb.dram.tile([1, W1TOT], f16)
        nc.gpsimd.dma_start(wpiece[:, :], WP[:, :])
        nc.gpsimd.collective_compute(
            "AllGather", OP.bypass, replica_groups=[[0, 2, 4, 6], [1, 3, 5, 7]],
            ins=[wpiece.opt()], outs=[wfull.opt()])
        w3piece = kb.dram.tile([1, W3TOT // 4], f16)
        w3full = kb.dram.tile([1, W3TOT], f16)
        nc.gpsimd.dma_start(w3piece[:, :], WP3Q[:, :])
        nc.gpsimd.collective_compute(
            "AllGather", OP.bypass, replica_groups=[[0, 2, 4, 6], [1, 3, 5, 7]],
            ins=[w3piece.opt()], outs=[w3full.opt()])
        nc.gpsimd.dma_start(WP3F[:, :], w3full[:, :])
        wq = kb.ld3(wfull, "wqT", 8, CH, W1OFF["wqT"], f16)
        wk = kb.ld3(wfull, "wkT", 8, CH, W1OFF["wkT"], f16)
        wv = kb.ld3(wfull, "wvT", 8, CH, W1OFF["wvT"], f16)
        wb = kb.ld3(wfull, "wbT", 8, 2, W1OFF["wbT"], f16)
        conv_t = {n: kb.ld3(CP, "conv" + n, 4, CONV, C1OFF["conv" + n])
                  for n in "qkv"}
        firs_t = kb.ld3(CP, "firs", 4, FIRS, C1OFF["firs"])
        firl_t = kb.ld3(CP, "firl", 4, FIRL, C1OFF["firl"])
        zpad = kb.const.tile([128, 64], f32)
        nc.vector.memset(zpad[:, :], 0.0)

        hid_half = kb.dram.tile([L // 2, HS], f16)
        hid = kb.dram.tile([L, HS], f16)
        raw = {n: kb.dram.tile([CH, L + 3], f32, tag="raw" + n, name="raw" + n)
               for n in "qkv"}
        nc.gpsimd.dma_start(hid_half[:, :], HH[:, :])
        nc.gpsimd.collective_compute(
            "AllGather", OP.bypass, replica_groups=PAIRS,
            ins=[hid_half.opt()], outs=[hid.opt()])
        nc.gpsimd.dma_start(HID16[:, :], hid[:, :])
        hidT = hid[:, :].rearrange("a b -> b a")

        for n in "qkv":
            for m in range(4):
                nc.sync.dma_start(out=raw[n][128 * m:128 * (m + 1), 0:3],
                                  in_=zpad[:, 0:3])
        for m in range(4):
            nc.sync.dma_start(out=VC[128 * m:128 * (m + 1), 0:63],
                              in_=zpad[:, 0:63])

        with tc.tile_pool(name="p1", bufs=2) as p1:
            for tb in range(NB):
                t0 = tb * 512
                ht = []
                for k in range(8):
                    h = p1.tile([128, 512], f16, tag=f"ht{k}", name=f"ht{k}")
                    nc.sync.dma_start(
                        out=h[:, :], in_=hidT[128 * k:128 * (k + 1), t0:t0 + 512])
                    ht.append(h)
                for (wt, dst) in ((wq, raw["q"]), (wk, raw["k"]), (wv, raw["v"])):
                    for m in range(4):
                        p = kb.psum([128, 512])
                        for k in range(8):
                            nc.tensor.matmul(p[:, :],
                                             wt[:, k, 128 * m:128 * (m + 1)],
                                             ht[k][:, :], start=(k == 0),
                                             stop=(k == 7))
                        sb = kb.scratch([128, 512])
                        nc.scalar.copy(out=sb[:, :], in_=p[:, :])
                        nc.sync.dma_start(
                            out=dst[128 * m:128 * (m + 1), 3 + t0:3 + t0 + 512],
                            in_=sb[:, :])
                pb = kb.psum([2, 512])
                for k in range(8):
                    nc.tensor.matmul(pb[:, :], wb[:, k, :], ht[k][:, :],
                                     start=(k == 0), stop=(k == 7))
                bsg = kb.scratch([2, 512])
                nc.scalar.activation(out=bsg[:, :], in_=pb[:, :], func=AF.Sigmoid)
                nc.sync.dma_start(out=BETA[:, t0:t0 + 512], in_=bsg[:, :])

        # conv + silu, full width
        with tc.tile_pool(name="p2", bufs=2) as p2:
            for n, dst, voff in (("q", QC, 0), ("k", KC, 0), ("v", VC, 63)):
                for m in range(4):
                    w = p2.tile([128, L + 3], f32, tag="convw", name="convw")
                    nc.sync.dma_start(out=w[:, :],
                                      in_=raw[n][128 * m:128 * (m + 1), :])
                    acc = p2.tile([128, L], f32, tag="convacc", name="convacc")
                    nc.vector.tensor_scalar(out=acc[:, :], in0=w[:, 0:L],
                                            scalar1=conv_t[n][:, m, 0:1],
                                            scalar2=None, op0=OP.mult)
                    for j in range(1, 4):
                        nc.vector.scalar_tensor_tensor(
                            out=acc[:, :], in0=w[:, j:j + L],
                            scalar=conv_t[n][:, m, j:j + 1], in1=acc[:, :],
                            op0=OP.mult, op1=OP.add)
                    sb = p2.tile([128, L], f32, tag="convout", name="convout")
                    nc.scalar.activation(out=sb[:, :], in_=acc[:, :], func=AF.Silu)
                    nc.sync.dma_start(
                        out=dst[128 * m:128 * (m + 1), voff:voff + L], in_=sb[:, :])
        # FIR, full width
        with tc.tile_pool(name="p3", bufs=2) as p3:
            for m in range(4):
                w = p3.tile([128, L + 63], f32, tag="firw", name="firw")
                nc.sync.dma_start(out=w[:, :], in_=VC[128 * m:128 * (m + 1), :])
                accl = p3.tile([128, L], f32, tag="firaccl", name="firaccl")
                nc.vector.tensor_scalar(out=accl[:, :], in0=w[:, 0:L],
                                        scalar1=firl_t[:, m, 0:1], scalar2=None,
                                        op0=OP.mult)
                for j in range(1, FIRL):
                    nc.vector.scalar_tensor_tensor(
                        out=accl[:, :], in0=w[:, j:j + L],
                        scalar=firl_t[:, m, j:j + 1], in1=accl[:, :],
                        op0=OP.mult, op1=OP.add)
                nc.sync.dma_start(out=FL[128 * m:128 * (m + 1), :], in_=accl[:, :])
                accs = p3.tile([128, L], f32, tag="firaccs", name="firaccs")
                nc.vector.tensor_scalar(out=accs[:, :], in0=w[:, 59:59 + L],
                                        scalar1=firs_t[:, m, 0:1], scalar2=None,
                                        op0=OP.mult)
                for j in range(1, FIRS):
                    nc.vector.scalar_tensor_tensor(
                        out=accs[:, :], in0=w[:, 59 + j:59 + j + L],
                        scalar=firs_t[:, m, j:j + 1], in1=accs[:, :],
                        op0=OP.mult, op1=OP.add)
                nc.sync.dma_start(out=FS[128 * m:128 * (m + 1), :], in_=accs[:, :])
        kb.ctx.close()
    nc.compile()
    return nc


# ================= K2 (per local head) =================
def build_k2(h):
    kb = KB()
    nc = kb.nc
    QC = nc.dram_tensor("qc", [CH, L], f32, kind="ExternalInput")
    KC = nc.dram_tensor("kc", [CH, L], f32, kind="ExternalInput")
    VC = nc.dram_tensor("vc", [CH, L + 63], f32, kind="ExternalInput")
    BETA = nc.dram_tensor("beta", [2, L], f32, kind="ExternalInput")
    CP = nc.dram_tensor("cpk2", [1, C2TOT], f32, kind="ExternalInput")
    ODC = nc.dram_tensor(f"odc{h}", [256, L], f32, kind="ExternalOutput")
    r0 = 256 * h
    with tile.TileContext(nc) as tc:
        kb.start(tc)
        iden = kb.ld2(CP, "iden", (128, 128), C2OFF["iden"])
        negU = kb.ld2(CP, "negU", (128, 128), C2OFF["negU"])
        uincl = kb.ld2(CP, "uincl", (128, 128), C2OFF["uincl"])
        ones = kb.ld2(CP, "ones", (128, 128), C2OFF["ones"])
        epsl2 = kb.const.tile([128, 1], f32)
        nc.vector.memset(epsl2[:, :], 1e-6)

        with tc.tile_pool(name="sp", bufs=1) as sp, \
             tc.tile_pool(name="stash", bufs=3) as stash, \
             tc.tile_pool(name="qkv", bufs=2) as qkv:
            S_sb = [sp.tile([128, 256], f32, tag=f"S{j}", name=f"S{j}")
                    for j in range(2)]
            for j in range(2):
                nc.vector.memset(S_sb[j][:, :], 0.0)
            for blk in range(NB):
                t0 = blk * 512
                qd, kd, vd = [], [], []
                for j in range(2):
                    rj = r0 + 128 * j
                    q_ = qkv.tile([128, 512], f32, tag=f"qd{j}", name=f"qd{j}")
                    nc.sync.dma_start(out=q_[:, :], in_=QC[rj:rj + 128, t0:t0 + 512])
                    qd.append(q_)
                    k_ = qkv.tile([128, 512], f32, tag=f"kd{j}", name=f"kd{j}")
                    nc.sync.dma_start(out=k_[:, :], in_=KC[rj:rj + 128, t0:t0 + 512])
                    kd.append(k_)
                    v_ = qkv.tile([128, 512], f32, tag=f"vd{j}", name=f"vd{j}")
                    nc.sync.dma_start(out=v_[:, :],
                                      in_=VC[rj:rj + 128, 63 + t0:63 + t0 + 512])
                    vd.append(v_)
                bb = qkv.tile([1, 512], f32, tag="bb", name="bb")
                nc.sync.dma_start(out=bb[:, :], in_=BETA[h:h + 1, t0:t0 + 512])
                rq_b = qkv.tile([1, 512], f32, tag="rqb", name="rqb")
                rk_b = qkv.tile([1, 512], f32, tag="rkb", name="rkb")
                for (dsrc, rdst) in ((qd, rq_b), (kd, rk_b)):
                    sqt = [kb.scratch([128, 512]) for _ in range(2)]
                    for j in range(2):
                        nc.scalar.activation(out=sqt[j][:, :], in_=dsrc[j][:, :],
                                             func=AF.Square)
                    pssum = kb.psum([1, 512])
                    for j in range(2):
                        nc.tensor.matmul(pssum[:, :], ones[:, 0:1], sqt[j][:, :],
                                         start=(j == 0), stop=(j == 1))
                    nrm = kb.scratch([1, 512])
                    nc.scalar.activation(out=nrm[:, :], in_=pssum[:, :],
                                         func=AF.Sqrt, bias=epsl2[0:1, :])
                    nc.vector.reciprocal(out=rdst[:, :], in_=nrm[:, :])
                for cc in range(4):
                    c0 = t0 + cc * 128
                    s0 = cc * 128
                    pcol = kb.psum([128, 3])
                    nc.tensor.matmul(pcol[:, 0:1], bb[0:1, s0:s0 + 128],
                                     ones[0:1, 0:1], start=True, stop=True)
                    nc.tensor.matmul(pcol[:, 1:2], rq_b[0:1, s0:s0 + 128],
                                     ones[0:1, 0:1], start=True, stop=True)
                    nc.tensor.matmul(pcol[:, 2:3], rk_b[0:1, s0:s0 + 128],
                                     ones[0:1, 0:1], start=True, stop=True)
                    cols = stash.tile([128, 3], f32, tag="cols", name="cols")
                    nc.vector.tensor_copy(out=cols[:, :], in_=pcol[:, :])
                    bcol, rqcol, rkcol = cols[:, 0:1], cols[:, 1:2], cols[:, 2:3]
                    brk = kb.scratch([1, 128])
                    nc.vector.tensor_tensor(out=brk[:, :],
                                            in0=bb[0:1, s0:s0 + 128],
                                            in1=rk_b[0:1, s0:s0 + 128], op=OP.mult)
                    prb = kb.psum([128, 128])
                    nc.tensor.matmul(prb[:, :], ones[0:1, :], brk[:, :],
                                     start=True, stop=True)
                    pg = kb.psum([128, 128])
                    for j in range(2):
                        nc.tensor.matmul(pg[:, :], kd[j][:, s0:s0 + 128],
                                         kd[j][:, s0:s0 + 128], start=(j == 0),
                                         stop=(j == 1))
                    ptmp = kb.scratch([128, 128])
                    nc.vector.scalar_tensor_tensor(out=ptmp[:, :], in0=pg[:, :],
                                                   scalar=rkcol, in1=negU[:, :],
                                                   op0=OP.mult, op1=OP.mult)
                    P_sb = kb.scratch([128, 128])
                    nc.vector.tensor_tensor(out=P_sb[:, :], in0=ptmp[:, :],
                                            in1=prb[:, :], op=OP.mult)
                    pa = kb.psum([128, 128])
                    for j in range(2):
                        nc.tensor.matmul(pa[:, :], kd[j][:, s0:s0 + 128],
                                         qd[j][:, s0:s0 + 128], start=(j == 0),
                                         stop=(j == 1))
                    attnT = stash.tile([128, 128], f32, tag="attnT", name="attnT")
                    nc.vector.scalar_tensor_tensor(out=attnT[:, :], in0=pa[:, :],
                                                   scalar=rkcol, in1=uincl[:, :],
                                                   op0=OP.mult, op1=OP.mult)
                    pkt = kb.psum([128, 256])
                    pvt = kb.psum([128, 256])
                    for j in range(2):
                        nc.tensor.matmul(pkt[:, 128 * j:128 * (j + 1)],
                                         kd[j][:, s0:s0 + 128], iden[:, :],
                                         start=True, stop=True)
                        nc.tensor.matmul(pvt[:, 128 * j:128 * (j + 1)],
                                         vd[j][:, s0:s0 + 128], iden[:, :],
                                         start=True, stop=True)
                    kntm = stash.tile([128, 256], f32, tag="kntm", name="kntm")
                    nc.vector.tensor_scalar(out=kntm[:, :], in0=pkt[:, :],
                                            scalar1=rkcol, scalar2=None,
                                            op0=OP.mult)
                    uw = stash.tile([128, 512], f32, tag="uw", name="uw")
                    nc.vector.tensor_scalar(out=uw[:, 0:256], in0=pvt[:, :],
                                            scalar1=bcol, scalar2=None,
                                            op0=OP.mult)
                    nc.vector.tensor_scalar(out=uw[:, 256:512], in0=kntm[:, :],
                                            scalar1=bcol, scalar2=None,
                                            op0=OP.mult)
                    for lvl in range(7):
                        puw = kb.psum([128, 512])
                        nc.tensor.matmul(puw[:, :], P_sb[:, :], uw[:, :],
                                         start=True, stop=True)
                        nc.vector.tensor_tensor(out=uw[:, :], in0=puw[:, :],
                                                in1=uw[:, :], op=OP.add)
                        if lvl < 6:
                            ptr = kb.psum([128, 128])
                            nc.tensor.matmul(ptr[:, :], P_sb[:, :], iden[:, :],
                                             start=True, stop=True)
                            PT_sb = kb.scratch([128, 128])
                            nc.vector.tensor_copy(out=PT_sb[:, :], in_=ptr[:, :])
                            psq = kb.psum([128, 128])
                            nc.tensor.matmul(psq[:, :], PT_sb[:, :], P_sb[:, :],
                                             start=True, stop=True)
                            P_sb = kb.scratch([128, 128])
                            nc.vector.tensor_copy(out=P_sb[:, :], in_=psq[:, :])
                    pwt = kb.psum([128, 256])
                    for j in range(2):
                        nc.tensor.matmul(pwt[:, 128 * j:128 * (j + 1)],
                                         uw[:, 256 + 128 * j:256 + 128 * (j + 1)],
                                         iden[:, :], start=True, stop=True)
                    wtcm = stash.tile([128, 256], f32, tag="wtcm", name="wtcm")
                    nc.vector.tensor_copy(out=wtcm[:, :], in_=pwt[:, :])

                    pws = kb.psum([128, 256])
                    for j in range(2):
                        nc.tensor.matmul(pws[:, :], wtcm[:, 128 * j:128 * (j + 1)],
                                         S_sb[j][:, :], start=(j == 0),
                                         stop=(j == 1))
                    ui = stash.tile([128, 256], f32, tag="ui", name="ui")
                    nc.vector.tensor_tensor(out=ui[:, :], in0=uw[:, 0:256],
                                            in1=pws[:, :], op=OP.subtract)
                    po = kb.psum([128, 256])
                    for j in range(2):
                        nc.tensor.matmul(po[:, :], qd[j][:, s0:s0 + 128],
                                         S_sb[j][:, :], start=(j == 0), stop=False)
                    nc.tensor.matmul(po[:, :], attnT[:, :], ui[:, :],
                                     start=False, stop=True)
                    otm = kb.scratch([128, 256])
                    nc.vector.tensor_scalar(out=otm[:, :], in0=po[:, :],
                                            scalar1=rqcol, scalar2=None,
                                            op0=OP.mult)
                    for j in range(2):
                        pds = kb.psum([128, 256])
                        nc.tensor.matmul(pds[:, :], kntm[:, 128 * j:128 * (j + 1)],
                                         ui[:, :], start=True, stop=True)
                        nc.vector.tensor_tensor(out=S_sb[j][:, :],
                                                in0=S_sb[j][:, :],
                                                in1=pds[:, :], op=OP.add)
                    pot = kb.psum([128, 256])
                    for j in range(2):
                        nc.tensor.matmul(pot[:, 128 * j:128 * (j + 1)],
                                         otm[:, 128 * j:128 * (j + 1)],
                                         iden[:, :], start=True, stop=True)
                    osb = kb.scratch([128, 256])
                    nc.scalar.copy(out=osb[:, :], in_=pot[:, :])
                    for j in range(2):
                        nc.sync.dma_start(
                            out=ODC[128 * j:128 * (j + 1), c0:c0 + 128],
                            in_=osb[:, 128 * j:128 * (j + 1)])
        kb.ctx.close()
    nc.compile()
    return nc


# ================= K3 (per time half) =================
def build_k3(p):
    kb = KB()
    nc = kb.nc
    HID16 = nc.dram_tensor("hid16", [L, HS], f16, kind="ExternalInput")
    VC = nc.dram_tensor("vc", [CH, L + 63], f32, kind="ExternalInput")
    FS = nc.dram_tensor("fs", [CH, L], f32, kind="ExternalInput")
    FL = nc.dram_tensor("fl", [CH, L], f32, kind="ExternalInput")
    OD0 = nc.dram_tensor("odc0", [256, L], f32, kind="ExternalInput")
    OD1 = nc.dram_tensor("odc1", [256, L], f32, kind="ExternalInput")
    WP = nc.dram_tensor("wpk3f", [1, W3TOT], f16, kind="ExternalInput")
    CP = nc.dram_tensor("cpk3", [1, C3TOT], f32, kind="ExternalInput")
    if p == 0:
        OUTP = nc.dram_tensor("outp0", [L // 2, HS], f32, kind="ExternalOutput")
    else:
        O0 = nc.dram_tensor("outp0", [L // 2, HS], f32, kind="ExternalInput")
        OUT = nc.dram_tensor("out", [L // 2, HS], f16, kind="ExternalOutput")
    with tile.TileContext(nc) as tc:
        kb.start(tc)
        if p == 1:
            outp_full = kb.dram.tile([L, HS], f32)
            outr = kb.dram.tile([L // 2, HS], f32)
            OUTP = outp_full[L // 2:L, :]
            nc.gpsimd.dma_start(outp_full[0:L // 2, :], O0[:, :])
        nc_ = nc
        ones = kb.ld2(CP, "ones", (128, 128), C3OFF["ones"])
        w1s = kb.ld2(CP, "w1sT", (16, HS), C3OFF["w1sT"], parts=32)
        b2s_t = kb.ld2(CP, "b2s", (4, 2), C3OFF["b2s"], parts=32)
        invt_t = kb.ld2(CP, "invt", (4, 2), C3OFF["invt"], parts=32)
        w2f = kb.ld3(CP, "w2T", 8, 4, C3OFF["w2T"])
        w2_t = kb.const.tile([128, 8, 4], bf16)
        nc.vector.tensor_copy(out=w2_t[:, :, :], in_=w2f[:, :, :])
        b1_t = kb.const.tile([128, 8], f32)
        nc.sync.dma_start(out=b1_t[:, :],
                          in_=_ap(CP, C3OFF["b1"], [[1, 128], [128, 8]]))
        normw_t = kb.const.tile([128, 2], f32)
        nc.sync.dma_start(out=normw_t[:, :],
                          in_=_ap(CP, C3OFF["normw"], [[1, 128], [128, 2]]))
        epsrms = kb.const.tile([128, 1], f32)
        nc.vector.memset(epsrms[:, :], RMS_EPS)
        w1 = kb.ld3(WP, "w1hT", 8, HS, W3OFF["w1hT"], f16)
        wo16 = kb.ld3(WP, "woT", 4, HS, W3OFF["woT"], f16)
        hidT = HID16[:, :].rearrange("a b -> b a")

        STATS_SRC = [("fs", FS, 0), ("fl", FL, 0), ("od", None, 0), ("v", VC, 63)]
        with tc.tile_pool(name="p7", bufs=2) as p7, \
             tc.tile_pool(name="p7one", bufs=1) as p7one, \
             tc.tile_pool(name="onp", bufs=2) as onp:
            for tbl in range(4):
                tb = 4 * p + tbl
                t0 = tb * 512
                tl0 = tbl * 512
                ht = []
                for k in range(8):
                    hh = p7.tile([128, 512], f16, tag=f"ht{k}", name=f"ht{k}")
                    nc.sync.dma_start(
                        out=hh[:, :], in_=hidT[128 * k:128 * (k + 1), t0:t0 + 512])
                    ht.append(hh)
                on_all = []
                for h in range(2):
                    r0 = 256 * h
                    odh = OD0 if h == 0 else OD1

                    def src_rows(ti, j):
                        nm, src, voff = STATS_SRC[ti]
                        if nm == "od":
                            return odh[128 * j:128 * (j + 1), t0:t0 + 512]
                        return src[r0 + 128 * j:r0 + 128 * (j + 1),
                                   voff + t0:voff + t0 + 512]

                    stats_blk = p7.tile([32, 512], f32, tag="statsblk",
                                        name="statsblk")
                    for ti in range(4):
                        bt = [p7.tile([128, 512], f32, tag=f"bt{j}", name=f"bt{j}")
                              for j in range(2)]
                        for j in range(2):
                            nc.sync.dma_start(out=bt[j][:, :], in_=src_rows(ti, j))
                        sq = [kb.scratch([128, 512]) for _ in range(2)]
                        ab = [kb.scratch([128, 512]) for _ in range(2)]
                        for j in range(2):
                            nc.scalar.activation(out=sq[j][:, :], in_=bt[j][:, :],
                                                 func=AF.Square)
                            nc.scalar.activation(out=ab[j][:, :], in_=bt[j][:, :],
                                                 func=AF.Abs)
                        psx = kb.psum([1, 512])
                        psq2 = kb.psum([1, 512])
                        psa = kb.psum([1, 512])
                        for j in range(2):
                            nc.tensor.matmul(psx[:, :], ones[:, 0:1], bt[j][:, :],
                                             start=(j == 0), stop=(j == 1))
                            nc.tensor.matmul(psq2[:, :], ones[:, 0:1], sq[j][:, :],
                                             start=(j == 0), stop=(j == 1))
                            nc.tensor.matmul(psa[:, :], ones[:, 0:1], ab[j][:, :],
                                             start=(j == 0), stop=(j == 1))
                        r = 4 * ti
                        fin = kb.scratch([1, 4, 512])
                        nc.scalar.activation(out=fin[:, 0, :], in_=psx[:, :],
                                             func=AF.Copy, scale=1.0 / 256.0)
                        m2 = kb.scratch([1, 512])
                        nc.vector.tensor_tensor(out=m2[:, :], in0=fin[:, 0, :],
                                                in1=fin[:, 0, :], op=OP.mult)
                        nc.vector.scalar_tensor_tensor(
                            out=fin[:, 1, :], in0=psq2[:, :], scalar=1.0 / 256.0,
                            in1=m2[:, :], op0=OP.mult, op1=OP.subtract)
                        nc.scalar.activation(out=fin[:, 2, :], in_=psa[:, :],
                                             func=AF.Copy, scale=1.0 / 256.0)
                        nc.scalar.activation(out=fin[:, 3, :], in_=psq2[:, :],
                                             func=AF.Sqrt)
                        nc.sync.dma_start(
                            out=stats_blk[r:r + 4, :],
                            in_=bass.AP(tensor=fin.tensor, offset=fin.offset,
                                        ap=[[512, 4], [1, 512]]))
                    # gate MLP
                    h1 = p7one.tile([128, 8, 512], bf16, tag="h1", name="h1")
                    for gm in range(8):
                        pg1 = kb.psum([128, 512])
                        for k in range(8):
                            nc.tensor.matmul(pg1[:, :],
                                             w1[:, k, 128 * gm:128 * (gm + 1)],
                                             ht[k][:, :], start=(k == 0),
                                             stop=False)
                        nc.tensor.matmul(pg1[:, :],
                                         w1s[0:16, 128 * gm:128 * (gm + 1)],
                                         stats_blk[0:16, :], start=False, stop=True)
                        nc.scalar.activation(out=h1[:, gm, :], in_=pg1[:, :],
                                             func=AF.Gelu_apprx_tanh,
                                             bias=b1_t[:, gm:gm + 1])
                    pl = kb.psum([4, 512])
                    for k in range(8):
                        nc.tensor.matmul(pl[:, :], w2_t[:, k, :], h1[:, k, :],
                                         start=(k == 0), stop=(k == 7))
                    el = kb.scratch([4, 512])
                    nc.scalar.activation(out=el[:, :], in_=pl[:, :], func=AF.Exp,
                                         scale=invt_t[0:4, h:h + 1],
                                         bias=b2s_t[0:4, h:h + 1])
                    pse = kb.psum([1, 512])
                    nc.tensor.matmul(pse[:, :], ones[0:4, 0:1], el[:, :],
                                     start=True, stop=True)
                    rec = kb.scratch([1, 512])
                    nc.vector.reciprocal(out=rec[:, :], in_=pse[:, :])
                    prr = kb.psum([4, 512])
                    nc.tensor.matmul(prr[:, :], ones[0:1, 0:4], rec[:, :],
                                     start=True, stop=True)
                    wgt = p7.tile([4, 512], f32, tag="wgtt", name="wgtt")
                    nc.vector.tensor_tensor(out=wgt[:, :], in0=el[:, :],
                                            in1=prr[:, :], op=OP.mult)
                    nc.vector.tensor_scalar(out=wgt[:, :], in0=wgt[:, :],
                                            scalar1=1.0 - 4.0 * EPS_FLOOR,
                                            scalar2=EPS_FLOOR, op0=OP.mult,
                                            op1=OP.add)
                    wbc = p7one.tile([128, 4, 512], f32, tag="wbc", name="wbc")
                    for cls in range(4):
                        wrow = kb.scratch([1, 512])
                        nc.sync.dma_start(out=wrow[:, :], in_=wgt[cls:cls + 1, :])
                        pwb = kb.psum([128, 512])
                        nc.tensor.matmul(pwb[:, :], ones[0:1, :], wrow[:, :],
                                         start=True, stop=True)
                        nc.scalar.copy(out=wbc[:, cls, :], in_=pwb[:, :])
                    ob = [p7one.tile([128, 512], f32, tag=f"ob{j}", name=f"ob{j}")
                          for j in range(2)]
                    osq = [kb.scratch([128, 512]) for _ in range(2)]
                    for cls in range(4):
                        for j in range(2):
                            bl = p7.tile([128, 512], f32, tag=f"bl{j}",
                                         name=f"bl{j}")
                            nc.sync.dma_start(out=bl[:, :], in_=src_rows(cls, j))
                            if cls == 0:
                                nc.vector.tensor_tensor(out=ob[j][:, :],
                                                        in0=bl[:, :],
                                                        in1=wbc[:, 0, :],
                                                        op=OP.mult)
                            else:
                                tbr = kb.scratch([128, 512])
                                nc.vector.tensor_tensor(out=tbr[:, :],
                                                        in0=bl[:, :],
                                                        in1=wbc[:, cls, :],
                                                        op=OP.mult)
                                nc.vector.tensor_tensor(out=ob[j][:, :],
                                                        in0=ob[j][:, :],
                                                        in1=tbr[:, :], op=OP.add)
                    for j in range(2):
                        nc.scalar.activation(out=osq[j][:, :], in_=ob[j][:, :],
                                             func=AF.Square)
                    pso = kb.psum([1, 512])
                    for j in range(2):
                        nc.tensor.matmul(pso[:, :], ones[:, 0:1], osq[j][:, :],
                                         start=(j == 0), stop=(j == 1))
                    srt = kb.scratch([1, 512])
                    nc.scalar.activation(out=srt[:, :], in_=pso[:, :],
                                         func=AF.Sqrt, scale=1.0 / 256.0,
                                         bias=epsrms[0:1, :])
                    rre = kb.scratch([1, 512])
                    nc.vector.reciprocal(out=rre[:, :], in_=srt[:, :])
                    prn = kb.psum([128, 512])
                    nc.tensor.matmul(prn[:, :], ones[0:1, :], rre[:, :],
                                     start=True, stop=True)
                    ons = []
                    for j in range(2):
                        on = onp.tile([128, 512], f16, tag=f"on{h}{j}",
                                      name=f"on{h}{j}")
                        nc.vector.scalar_tensor_tensor(
                            out=on[:, :], in0=ob[j][:, :],
                            scalar=normw_t[:, j:j + 1], in1=prn[:, :],
                            op0=OP.mult, op1=OP.mult)
                        ons.append(on)
                    on_all.append(ons)
                for m in range(8):
                    pw = kb.psum([128, 512])
                    for d in range(4):
                        nc.tensor.matmul(pw[:, :],
                                         wo16[:, d, 128 * m:128 * (m + 1)],
                                         on_all[d // 2][d % 2][:, :],
                                         start=(d == 0), stop=(d == 3))
                    owr = kb.scratch([128, 512])
                    nc.vector.tensor_copy(out=owr[:, :], in_=pw[:, :])
                    nc.sync.dma_start(
                        out=OUTP[tl0:tl0 + 512,
                                 128 * m:128 * (m + 1)].rearrange("a b -> b a"),
                        in_=owr[:, :])
        if p == 1:
            nc.gpsimd.collective_compute(
                "ReduceScatter", OP.add, replica_groups=PAIRS,
                ins=[outp_full.opt()], outs=[outr.opt()])
            with tc.tile_pool(name="p10", bufs=2) as p10:
                for r in range(16):
                    for half in range(2):
                        t = p10.tile([128, 512], f32, tag="oload", name="oload")
                        nc.sync.dma_start(
                            out=t[:, :],
                            in_=outr[128 * r:128 * (r + 1),
                                     512 * half:512 * (half + 1)])
                        t16 = p10.tile([128, 512], f16, tag="o16", name="o16")
                        nc.vector.tensor_copy(out=t16[:, :], in_=t[:, :])
                        nc.sync.dma_start(
                            out=OUT[128 * r:128 * (r + 1),
                                    512 * half:512 * (half + 1)],
                            in_=t16[:, :])
        kb.ctx.close()
    nc.compile()
    return nc


# ================= runner =================
def make_jit(nc, mesh):
    install_neuronx_cc_hook()
    pname = nc.partition_id_tensor.name if nc.partition_id_tensor else None
    in_names, out_names, out_avals = [], [], []
    for alloc in nc.m.functions[0].allocations:
        if not isinstance(alloc, mybir.MemoryLocationSet):
            continue
        name = alloc.memorylocations[0].name
        if alloc.kind == "ExternalInput":
            if name != pname:
                in_names.append(name)
        elif alloc.kind == "ExternalOutput":
            out_names.append(name)
            out_avals.append(jax.core.ShapedArray(
                tuple(alloc.tensor_shape), mybir.dt.np(alloc.dtype)))
    n_params = len(in_names)
    all_names = list(in_names)
    if pname is not None:
        all_names.append(pname)
    all_names = tuple(all_names)

    def _body(*args):
        operands = list(args)
        if pname is not None:
            operands.append(partition_id_tensor())
        outs = _bass_exec_p.bind(
            *operands, out_avals=tuple(out_avals), in_names=all_names,
            out_names=tuple(out_names), lowering_input_output_aliases=(),
            sim_require_finite=True, sim_require_nnan=True, nc=nc)
        return tuple(outs)

    P = PartitionSpec
    fn = jax.jit(
        shard_map(_body, mesh=mesh,
                  in_specs=(P("core"),) * n_params,
                  out_specs=(P("core"),) * len(out_names), check_rep=False),
        keep_unused=True)
    return fn, in_names, out_names, out_avals


class Chain:
    def __init__(self, ncs):
        self.mesh = Mesh(np.asarray(jax.devices()[:8]), ("core",))
        self.jits = [make_jit(nc, self.mesh) for nc in ncs]

    def run(self, host_inputs):
        """host_inputs: dict name -> np array (8*rows, cols). Returns bufs."""
        bufs = dict(host_inputs)
        for fn, in_names, out_names, out_avals in self.jits:
            args = [bufs[n] for n in in_names]
            outs = fn(*args)
            bufs.update(zip(out_names, outs))
        return bufs


_CHAIN = None


def get_chain():
    global _CHAIN
    if _CHAIN is None:
        ncs = [build_k1(), build_k2(0), build_k2(1), build_k3(0), build_k3(1)]
        _CHAIN = Chain(ncs)
    return _CHAIN


# ================= host packing =================
def pack_inputs(inputs):
    hidden = np.asarray(inputs["hidden_states"], np.float32)
    Wq = np.asarray(inputs["Wq"], np.float32)
    Wk = np.asarray(inputs["Wk"], np.float32)
    Wv = np.asarray(inputs["Wv"], np.float32)
    Wb = np.asarray(inputs["Wb"], np.float32)
    W1 = np.asarray(inputs["gate_W1"], np.float32)
    W2 = np.asarray(inputs["gate_W2"], np.float32)
    b1 = np.asarray(inputs["gate_b1"], np.float32)
    b2 = np.asarray(inputs["gate_b2"], np.float32)
    cpb = np.asarray(inputs["gate_copy_bias"], np.float32)
    ltp = np.asarray(inputs["gate_log_temp"], np.float32)
    Wo = np.asarray(inputs["Wo"], np.float32)
    normw = np.asarray(inputs["o_norm_w"], np.float32)
    cq = np.asarray(inputs["conv_q_w"], np.float32)
    ck = np.asarray(inputs["conv_k_w"], np.float32)
    cv = np.asarray(inputs["conv_v_w"], np.float32)
    firs = np.asarray(inputs["fir_short_filt"], np.float32).reshape(NH * DV, FIRS)
    firl = np.asarray(inputs["fir_long_filt"], np.float32).reshape(NH * DV, FIRL)

    iden = np.eye(128, dtype=np.float32)
    a = np.arange(128)
    negU = np.where(a[:, None] < a[None, :], -1.0, 0.0).astype(np.float32)
    uincl = np.where(a[:, None] <= a[None, :], 1.0, 0.0).astype(np.float32)
    onesm = np.ones((128, 128), np.float32)

    def pk(spec, offs, tot, parts, dt):
        buf = np.empty(tot, dt)
        for nm, shp in spec:
            buf[offs[nm]:offs[nm] + int(np.prod(shp))] = \
                parts[nm].astype(dt).ravel()
        return buf.reshape(1, -1)

    per = {k: [] for k in ("hhalf", "wpk1", "cpk1", "cpk2", "wpk3q", "cpk3")}
    for c in range(8):
        b, hl = c // 2, c % 2
        rows = slice(512 * hl, 512 * (hl + 1))
        heads = [2 * hl, 2 * hl + 1]
        invt = np.exp(-ltp[heads])
        b2eff = np.stack([b2 + np.array([0, 0, 0, cpb[hh] * DECAY], np.float32)
                          for hh in heads], 1)
        per["hhalf"].append(np.ascontiguousarray(
            hidden[b, 2048 * hl:2048 * (hl + 1), :].astype(np.float16)))
        w1flat = pk(WPK1, W1OFF, W1TOT,
                    {"wqT": Wq[rows].T, "wkT": Wk[rows].T,
                     "wvT": Wv[rows].T, "wbT": Wb[heads].T}, np.float16)
        qtr = W1TOT // 4
        per["wpk1"].append(w1flat[:, (c // 2) * qtr:(c // 2 + 1) * qtr])
        per["cpk1"].append(pk(CPK1, C1OFF, C1TOT,
                              {"convq": cq[rows], "convk": ck[rows],
                               "convv": cv[rows], "firs": firs[rows],
                               "firl": firl[rows]}, np.float32))
        per["cpk2"].append(pk(CPK2, C2OFF, C2TOT,
                              {"iden": iden, "negU": negU, "uincl": uincl,
                               "ones": onesm}, np.float32))
        w3flat = pk(WPK3, W3OFF, W3TOT,
                    {"w1hT": W1[:, :HS].T, "woT": Wo[:, rows].T}, np.float16)
        qtr3 = W3TOT // 4
        per["wpk3q"].append(w3flat[:, (c // 2) * qtr3:(c // 2 + 1) * qtr3])
        per["cpk3"].append(pk(CPK3, C3OFF, C3TOT,
                              {"ones": onesm, "w1sT": W1[:, HS:HS + 16].T,
                               "w2T": W2.T, "b1": b1.reshape(HS, 1),
                               "normw": normw.reshape(DV, 1),
                               "b2s": b2eff * invt[None, :],
                               "invt": np.broadcast_to(invt[None, :], (4, 2))},
                              np.float32))
    return {k: np.concatenate(v, axis=0) for k, v in per.items()}


def unpack_output(out_global):
    o = np.asarray(out_global).reshape(8, 2048, HS)
    out = np.empty((B, L, HS), np.float32)
    for c in range(8):
        b, hl = c // 2, c % 2
        out[b, 2048 * hl:2048 * (hl + 1), :] = o[c]
    return out


def kernel(**inputs):
    chain = get_chain()
    host = pack_inputs(inputs)
    bufs = chain.run(host)
    return unpack_output(bufs["out"])


# ================= warm-load at import =================
def _warmup():
    chain = get_chain()
    host = {
        "hhalf": np.zeros((8 * 2048, HS), np.float16),
        "wpk1": np.zeros((8, W1TOT // 4), np.float16),
        "cpk1": np.zeros((8, C1TOT), np.float32),
        "cpk2": np.zeros((8, C2TOT), np.float32),
        "wpk3q": np.zeros((8, W3TOT // 4), np.float16),
        "cpk3": np.zeros((8, C3TOT), np.float32),
    }
    bufs = chain.run(host)
    np.asarray(bufs["out"])
    return chain


try:
    _warmup()
except Exception:
    _CHAIN = None  # fall back to building lazily inside kernel()


# revision 5
# speedup vs baseline: 47.4116x; 1.0299x over previous
"""nn_DeltaNet_31877247271467 — fully-fused TRN2 Bass kernel (8 NeuronCores).

Sharding: core c = (batch c//2, head-half c%2).  hidden/output move host<->device
as fp16; all device compute is fp32 (fp16 operands feed the big matmuls).
The forward pass runs as 5 small chained NEFFs with device-resident
intermediates.  Collectives: pair AllGather for hidden, group-of-4 AllGathers
(in K1) deduplicating both weight packs, pair ReduceScatter for the output
projection partials.  NEFFs are built, compiled, and warm-loaded at import.
"""

import sys

sys.path.insert(0, "/opt/trn_rl_repo")

import numpy as np
from contextlib import ExitStack

import jax
import concourse.bass as bass
import concourse.bacc as bacc
import concourse.tile as tile
from concourse import mybir
from concourse.bass2jax import _bass_exec_p, install_neuronx_cc_hook, partition_id_tensor
from jax.experimental.shard_map import shard_map
from jax.sharding import Mesh, PartitionSpec

f32, f16, bf16 = mybir.dt.float32, mybir.dt.float16, mybir.dt.bfloat16
AF = mybir.ActivationFunctionType
OP = mybir.AluOpType

B, L, HS = 4, 4096, 1024
NH, DK, DV = 4, 256, 256
CONV, FIRS, FIRL = 4, 5, 64
DECAY = 1.0 - 1.0 / 3000.0
EPS_FLOOR = 0.08 * DECAY
RMS_EPS = 1e-05
CH = 512
NB = L // 512
PAIRS = [[0, 1], [2, 3], [4, 5], [6, 7]]

WPK1 = [("wqT", (HS, CH)), ("wkT", (HS, CH)), ("wvT", (HS, CH)), ("wbT", (HS, 2))]
CPK1 = [("convq", (CH, CONV)), ("convk", (CH, CONV)), ("convv", (CH, CONV)),
        ("firs", (CH, FIRS)), ("firl", (CH, FIRL))]
CPK2 = [("iden", (128, 128)), ("negU", (128, 128)), ("uincl", (128, 128)),
        ("ones", (128, 128))]
WPK3 = [("w1hT", (HS, HS)), ("woT", (CH, HS))]
CPK3 = [("ones", (128, 128)), ("w1sT", (16, HS)), ("w2T", (HS, 4)),
        ("b1", (HS, 1)), ("normw", (DV, 1)), ("b2s", (4, 2)), ("invt", (4, 2))]


def _offsets(spec):
    offs, o = {}, 0
    for name, shp in spec:
        offs[name] = o
        o += int(np.prod(shp))
    return offs, o


W1OFF, W1TOT = _offsets(WPK1)
C1OFF, C1TOT = _offsets(CPK1)
C2OFF, C2TOT = _offsets(CPK2)
W3OFF, W3TOT = _offsets(WPK3)
C3OFF, C3TOT = _offsets(CPK3)


def _ap(dram_t, off, ap):
    return bass.AP(tensor=dram_t[0, :].tensor, offset=off, ap=ap)


class KB:
    """Shared per-kernel build helpers."""

    def __init__(self):
        self.nc = bacc.Bacc("TRN2", target_bir_lowering=False, debug=False,
                            num_devices=8)

    def start(self, tc):
        self.tc = tc
        self.ctx = ExitStack()
        self.const = self.ctx.enter_context(tc.tile_pool(name="const", bufs=1))
        self.dram = self.ctx.enter_context(
            tc.tile_pool(name="dram", bufs=1, space="DRAM"))
        self.ps = self.ctx.enter_context(
            tc.tile_pool(name="ps", bufs=7, space="PSUM"))
        self.sc = self.ctx.enter_context(tc.tile_pool(name="sc", bufs=10))

    def psum(self, shape):
        return self.ps.tile(shape, f32, tag="pp", name="pp")

    def scratch(self, shape, dt=f32):
        return self.sc.tile(shape, dt, tag="s512", name="s512")

    def ld2(self, src_dram, name, shape, off, dt=f32, parts=None):
        p = parts or shape[0]
        t = self.const.tile([p, shape[1]], dt, tag="c_" + name, name="c_" + name)
        self.nc.sync.dma_start(out=t[:shape[0], :],
                               in_=_ap(src_dram, off,
                                       [[shape[1], shape[0]], [1, shape[1]]]))
        return t

    def ld3(self, src_dram, name, ktiles, inner, off, dt=f32):
        t = self.const.tile([128, ktiles, inner], dt, tag="c3_" + name,
                            name="c3_" + name)
        self.nc.sync.dma_start(
            out=t[:, :, :],
            in_=_ap(src_dram, off,
                    [[inner, 128], [128 * inner, ktiles], [1, inner]]))
        return t


# ================= K1 =================
def build_k1():
    kb = KB()
    nc = kb.nc
    HH = nc.dram_tensor("hhalf", [L // 2, HS], f16, kind="ExternalInput")
    WP = nc.dram_tensor("wpk1", [1, W1TOT // 4], f16, kind="ExternalInput")
    CP = nc.dram_tensor("cpk1", [1, C1TOT], f32, kind="ExternalInput")
    WP3Q = nc.dram_tensor("wpk3q", [1, W3TOT // 4], f16, kind="ExternalInput")
    WP3F = nc.dram_tensor("wpk3f", [1, W3TOT], f16, kind="ExternalOutput")
    HID16 = nc.dram_tensor("hid16", [L, HS], f16, kind="ExternalOutput")
    QC = nc.dram_tensor("qc", [CH, L], f32, kind="ExternalOutput")
    KC = nc.dram_tensor("kc", [CH, L], f32, kind="ExternalOutput")
    VC = nc.dram_tensor("vc", [CH, L + 63], f32, kind="ExternalOutput")
    FS = nc.dram_tensor("fs", [CH, L], f32, kind="ExternalOutput")
    FL = nc.dram_tensor("fl", [CH, L], f32, kind="ExternalOutput")
    BETA = nc.dram_tensor("beta", [2, L], f32, kind="ExternalOutput")
    with tile.TileContext(nc) as tc:
        kb.start(tc)
        wpiece = kb.dram.tile([1, W1TOT // 4], f16)
        wfull = kb.dram.tile([1, W1TOT], f16)
        nc.gpsimd.dma_start(wpiece[:, :], WP[:, :])
        nc.gpsimd.collective_compute(
            "AllGather", OP.bypass, replica_groups=[[0, 2, 4, 6], [1, 3, 5, 7]],
            ins=[wpiece.opt()], outs=[wfull.opt()])
        w3piece = kb.dram.tile([1, W3TOT // 4], f16)
        w3full = kb.dram.tile([1, W3TOT], f16)
        nc.gpsimd.dma_start(w3piece[:, :], WP3Q[:, :])
        nc.gpsimd.collective_compute(
            "AllGather", OP.bypass, replica_groups=[[0, 2, 4, 6], [1, 3, 5, 7]],
            ins=[w3piece.opt()], outs=[w3full.opt()])
        nc.gpsimd.dma_start(WP3F[:, :], w3full[:, :])
        wq = kb.ld3(wfull, "wqT", 8, CH, W1OFF["wqT"], f16)
        wk = kb.ld3(wfull, "wkT", 8, CH, W1OFF["wkT"], f16)
        wv = kb.ld3(wfull, "wvT", 8, CH, W1OFF["wvT"], f16)
        wb = kb.ld3(wfull, "wbT", 8, 2, W1OFF["wbT"], f16)
        conv_t = {n: kb.ld3(CP, "conv" + n, 4, CONV, C1OFF["conv" + n])
                  for n in "qkv"}
        firs_t = kb.ld3(CP, "firs", 4, FIRS, C1OFF["firs"])
        firl_t = kb.ld3(CP, "firl", 4, FIRL, C1OFF["firl"])
        zpad = kb.const.tile([128, 64], f32)
        nc.vector.memset(zpad[:, :], 0.0)

        hid_half = kb.dram.tile([L // 2, HS], f16)
        hid = kb.dram.tile([L, HS], f16)
        raw = {n: kb.dram.tile([CH, L + 3], f32, tag="raw" + n, name="raw" + n)
               for n in "qkv"}
        nc.gpsimd.dma_start(hid_half[:, :], HH[:, :])
        nc.gpsimd.collective_compute(
            "AllGather", OP.bypass, replica_groups=PAIRS,
            ins=[hid_half.opt()], outs=[hid.opt()])
        nc.gpsimd.dma_start(HID16[:, :], hid[:, :])
        hidT = hid[:, :].rearrange("a b -> b a")

        for n in "qkv":
            for m in range(4):
                nc.sync.dma_start(out=raw[n][128 * m:128 * (m + 1), 0:3],
                                  in_=zpad[:, 0:3])
        for m in range(4):
            nc.sync.dma_start(out=VC[128 * m:128 * (m + 1), 0:63],
                              in_=zpad[:, 0:63])

        with tc.tile_pool(name="p1", bufs=2) as p1:
            for tb in range(NB):
                t0 = tb * 512
                ht = []
                for k in range(8):
                    h = p1.tile([128, 512], f16, tag=f"ht{k}", name=f"ht{k}")
                    nc.sync.dma_start(
                        out=h[:, :], in_=hidT[128 * k:128 * (k + 1), t0:t0 + 512])
                    ht.append(h)
                for (wt, dst) in ((wq, raw["q"]), (wk, raw["k"]), (wv, raw["v"])):
                    for m in range(4):
                        p = kb.psum([128, 512])
                        for k in range(8):
                            nc.tensor.matmul(p[:, :],
                                             wt[:, k, 128 * m:128 * (m + 1)],
                                             ht[k][:, :], start=(k == 0),
                                             stop=(k == 7))
                        sb = kb.scratch([128, 512])
                        nc.scalar.copy(out=sb[:, :], in_=p[:, :])
                        nc.sync.dma_start(
                            out=dst[128 * m:128 * (m + 1), 3 + t0:3 + t0 + 512],
                            in_=sb[:, :])
                pb = kb.psum([2, 512])
                for k in range(8):
                    nc.tensor.matmul(pb[:, :], wb[:, k, :], ht[k][:, :],
                                     start=(k == 0), stop=(k == 7))
                bsg = kb.scratch([2, 512])
                nc.scalar.activation(out=bsg[:, :], in_=pb[:, :], func=AF.Sigmoid)
                nc.sync.dma_start(out=BETA[:, t0:t0 + 512], in_=bsg[:, :])

        # conv + silu, full width
        with tc.tile_pool(name="p2", bufs=2) as p2:
            for n, dst, voff in (("q", QC, 0), ("k", KC, 0), ("v", VC, 63)):
                for m in range(4):
                    w = p2.tile([128, L + 3], f32, tag="convw", name="convw")
                    nc.sync.dma_start(out=w[:, :],
                                      in_=raw[n][128 * m:128 * (m + 1), :])
                    acc = p2.tile([128, L], f32, tag="convacc", name="convacc")
                    nc.vector.tensor_scalar(out=acc[:, :], in0=w[:, 0:L],
                                            scalar1=conv_t[n][:, m, 0:1],
                                            scalar2=None, op0=OP.mult)
                    for j in range(1, 4):
                        nc.vector.scalar_tensor_tensor(
                            out=acc[:, :], in0=w[:, j:j + L],
                            scalar=conv_t[n][:, m, j:j + 1], in1=acc[:, :],
                            op0=OP.mult, op1=OP.add)
                    sb = p2.tile([128, L], f32, tag="convout", name="convout")
                    nc.scalar.activation(out=sb[:, :], in_=acc[:, :], func=AF.Silu)
                    nc.sync.dma_start(
                        out=dst[128 * m:128 * (m + 1), voff:voff + L], in_=sb[:, :])
        # FIR, full width
        with tc.tile_pool(name="p3", bufs=2) as p3:
            for m in range(4):
                w = p3.tile([128, L + 63], f32, tag="firw", name="firw")
                nc.sync.dma_start(out=w[:, :], in_=VC[128 * m:128 * (m + 1), :])
                accl = p3.tile([128, L], f32, tag="firaccl", name="firaccl")
                nc.vector.tensor_scalar(out=accl[:, :], in0=w[:, 0:L],
                                        scalar1=firl_t[:, m, 0:1], scalar2=None,
                                        op0=OP.mult)
                for j in range(1, FIRL):
                    nc.vector.scalar_tensor_tensor(
                        out=accl[:, :], in0=w[:, j:j + L],
                        scalar=firl_t[:, m, j:j + 1], in1=accl[:, :],
                        op0=OP.mult, op1=OP.add)
                nc.sync.dma_start(out=FL[128 * m:128 * (m + 1), :], in_=accl[:, :])
                accs = p3.tile([128, L], f32, tag="firaccs", name="firaccs")
                nc.vector.tensor_scalar(out=accs[:, :], in0=w[:, 59:59 + L],
                                        scalar1=firs_t[:, m, 0:1], scalar2=None,
                                        op0=OP.mult)
                for j in range(1, FIRS):
                    nc.vector.scalar_tensor_tensor(
                        out=accs[:, :], in0=w[:, 59 + j:59 + j + L],
                        scalar=firs_t[:, m, j:j + 1], in1=accs[:, :],
                        op0=OP.mult, op1=OP.add)
                nc.sync.dma_start(out=FS[128 * m:128 * (m + 1), :], in_=accs[:, :])
        kb.ctx.close()
    nc.compile()
    return nc


# ================= K2 (per local head) =================
def build_k2(h):
    kb = KB()
    nc = kb.nc
    QC = nc.dram_tensor("qc", [CH, L], f32, kind="ExternalInput")
    KC = nc.dram_tensor("kc", [CH, L], f32, kind="ExternalInput")
    VC = nc.dram_tensor("vc", [CH, L + 63], f32, kind="ExternalInput")
    BETA = nc.dram_tensor("beta", [2, L], f32, kind="ExternalInput")
    CP = nc.dram_tensor("cpk2", [1, C2TOT], f32, kind="ExternalInput")
    ODC = nc.dram_tensor(f"odc{h}", [256, L], f32, kind="ExternalOutput")
    r0 = 256 * h
    with tile.TileContext(nc) as tc:
        kb.start(tc)
        iden = kb.ld2(CP, "iden", (128, 128), C2OFF["iden"])
        negU = kb.ld2(CP, "negU", (128, 128), C2OFF["negU"])
        uincl = kb.ld2(CP, "uincl", (128, 128), C2OFF["uincl"])
        ones = kb.ld2(CP, "ones", (128, 128), C2OFF["ones"])
        epsl2 = kb.const.tile([128, 1], f32)
        nc.vector.memset(epsl2[:, :], 1e-6)

        with tc.tile_pool(name="sp", bufs=1) as sp, \
             tc.tile_pool(name="stash", bufs=3) as stash, \
             tc.tile_pool(name="qkv", bufs=2) as qkv:
            S_sb = [sp.tile([128, 256], f32, tag=f"S{j}", name=f"S{j}")
                    for j in range(2)]
            for j in range(2):
                nc.vector.memset(S_sb[j][:, :], 0.0)
            for blk in range(NB):
                t0 = blk * 512
                qd, kd, vd = [], [], []
                for j in range(2):
                    rj = r0 + 128 * j
                    q_ = qkv.tile([128, 512], f32, tag=f"qd{j}", name=f"qd{j}")
                    nc.sync.dma_start(out=q_[:, :], in_=QC[rj:rj + 128, t0:t0 + 512])
                    qd.append(q_)
                    k_ = qkv.tile([128, 512], f32, tag=f"kd{j}", name=f"kd{j}")
                    nc.sync.dma_start(out=k_[:, :], in_=KC[rj:rj + 128, t0:t0 + 512])
                    kd.append(k_)
                    v_ = qkv.tile([128, 512], f32, tag=f"vd{j}", name=f"vd{j}")
                    nc.sync.dma_start(out=v_[:, :],
                                      in_=VC[rj:rj + 128, 63 + t0:63 + t0 + 512])
                    vd.append(v_)
                bb = qkv.tile([1, 512], f32, tag="bb", name="bb")
                nc.sync.dma_start(out=bb[:, :], in_=BETA[h:h + 1, t0:t0 + 512])
                rq_b = qkv.tile([1, 512], f32, tag="rqb", name="rqb")
                rk_b = qkv.tile([1, 512], f32, tag="rkb", name="rkb")
                for (dsrc, rdst) in ((qd, rq_b), (kd, rk_b)):
                    sqt = [kb.scratch([128, 512]) for _ in range(2)]
                    for j in range(2):
                        nc.scalar.activation(out=sqt[j][:, :], in_=dsrc[j][:, :],
                                             func=AF.Square)
                    pssum = kb.psum([1, 512])
                    for j in range(2):
                        nc.tensor.matmul(pssum[:, :], ones[:, 0:1], sqt[j][:, :],
                                         start=(j == 0), stop=(j == 1))
                    nrm = kb.scratch([1, 512])
                    nc.scalar.activation(out=nrm[:, :], in_=pssum[:, :],
                                         func=AF.Sqrt, bias=epsl2[0:1, :])
                    nc.vector.reciprocal(out=rdst[:, :], in_=nrm[:, :])
                for cc in range(4):
                    c0 = t0 + cc * 128
                    s0 = cc * 128
                    pcol = kb.psum([128, 3])
                    nc.tensor.matmul(pcol[:, 0:1], bb[0:1, s0:s0 + 128],
                                     ones[0:1, 0:1], start=True, stop=True)
                    nc.tensor.matmul(pcol[:, 1:2], rq_b[0:1, s0:s0 + 128],
                                     ones[0:1, 0:1], start=True, stop=True)
                    nc.tensor.matmul(pcol[:, 2:3], rk_b[0:1, s0:s0 + 128],
                                     ones[0:1, 0:1], start=True, stop=True)
                    cols = stash.tile([128, 3], f32, tag="cols", name="cols")
                    nc.vector.tensor_copy(out=cols[:, :], in_=pcol[:, :])
                    bcol, rqcol, rkcol = cols[:, 0:1], cols[:, 1:2], cols[:, 2:3]
                    brk = kb.scratch([1, 128])
                    nc.vector.tensor_tensor(out=brk[:, :],
                                            in0=bb[0:1, s0:s0 + 128],
                                            in1=rk_b[0:1, s0:s0 + 128], op=OP.mult)
                    prb = kb.psum([128, 128])
                    nc.tensor.matmul(prb[:, :], ones[0:1, :], brk[:, :],
                                     start=True, stop=True)
                    pg = kb.psum([128, 128])
                    for j in range(2):
                        nc.tensor.matmul(pg[:, :], kd[j][:, s0:s0 + 128],
                                         kd[j][:, s0:s0 + 128], start=(j == 0),
                                         stop=(j == 1))
                    ptmp = kb.scratch([128, 128])
                    nc.vector.scalar_tensor_tensor(out=ptmp[:, :], in0=pg[:, :],
                                                   scalar=rkcol, in1=negU[:, :],
                                                   op0=OP.mult, op1=OP.mult)
                    P_sb = kb.scratch([128, 128])
                    nc.vector.tensor_tensor(out=P_sb[:, :], in0=ptmp[:, :],
                                            in1=prb[:, :], op=OP.mult)
                    pa = kb.psum([128, 128])
                    for j in range(2):
                        nc.tensor.matmul(pa[:, :], kd[j][:, s0:s0 + 128],
                                         qd[j][:, s0:s0 + 128], start=(j == 0),
                                         stop=(j == 1))
                    attnT = stash.tile([128, 128], f32, tag="attnT", name="attnT")
                    nc.vector.scalar_tensor_tensor(out=attnT[:, :], in0=pa[:, :],
                                                   scalar=rkcol, in1=uincl[:, :],
                                                   op0=OP.mult, op1=OP.mult)
                    pkt = kb.psum([128, 256])
                    pvt = kb.psum([128, 256])
                    for j in range(2):
                        nc.tensor.matmul(pkt[:, 128 * j:128 * (j + 1)],
                                         kd[j][:, s0:s0 + 128], iden[:, :],
                                         start=True, stop=True)
                        nc.tensor.matmul(pvt[:, 128 * j:128 * (j + 1)],
                                         vd[j][:, s0:s0 + 128], iden[:, :],
                                         start=True, stop=True)
                    kntm = stash.tile([128, 256], f32, tag="kntm", name="kntm")
                    nc.vector.tensor_scalar(out=kntm[:, :], in0=pkt[:, :],
                                            scalar1=rkcol, scalar2=None,
                                            op0=OP.mult)
                    uw = stash.tile([128, 512], f32, tag="uw", name="uw")
                    nc.vector.tensor_scalar(out=uw[:, 0:256], in0=pvt[:, :],
                                            scalar1=bcol, scalar2=None,
                                            op0=OP.mult)
                    nc.vector.tensor_scalar(out=uw[:, 256:512], in0=kntm[:, :],
                                            scalar1=bcol, scalar2=None,
                                            op0=OP.mult)
                    for lvl in range(7):
                        puw = kb.psum([128, 512])
                        nc.tensor.matmul(puw[:, :], P_sb[:, :], uw[:, :],
                                         start=True, stop=True)
                        nc.vector.tensor_tensor(out=uw[:, :], in0=puw[:, :],
                                                in1=uw[:, :], op=OP.add)
                        if lvl < 6:
                            ptr = kb.psum([128, 128])
                            nc.tensor.matmul(ptr[:, :], P_sb[:, :], iden[:, :],
                                             start=True, stop=True)
                            PT_sb = kb.scratch([128, 128])
                            nc.vector.tensor_copy(out=PT_sb[:, :], in_=ptr[:, :])
                            psq = kb.psum([128, 128])
                            nc.tensor.matmul(psq[:, :], PT_sb[:, :], P_sb[:, :],
                                             start=True, stop=True)
                            P_sb = kb.scratch([128, 128])
                            nc.vector.tensor_copy(out=P_sb[:, :], in_=psq[:, :])
                    pwt = kb.psum([128, 256])
                    for j in range(2):
                        nc.tensor.matmul(pwt[:, 128 * j:128 * (j + 1)],
                                         uw[:, 256 + 128 * j:256 + 128 * (j + 1)],
                                         iden[:, :], start=True, stop=True)
                    wtcm = stash.tile([128, 256], f32, tag="wtcm", name="wtcm")
                    nc.vector.tensor_copy(out=wtcm[:, :], in_=pwt[:, :])

                    pws = kb.psum([128, 256])
                    for j in range(2):
                        nc.tensor.matmul(pws[:, :], wtcm[:, 128 * j:128 * (j + 1)],
                                         S_sb[j][:, :], start=(j == 0),
                                         stop=(j == 1))
                    ui = stash.tile([128, 256], f32, tag="ui", name="ui")
                    nc.vector.tensor_tensor(out=ui[:, :], in0=uw[:, 0:256],
                                            in1=pws[:, :], op=OP.subtract)
                    po = kb.psum([128, 256])
                    for j in range(2):
                        nc.tensor.matmul(po[:, :], qd[j][:, s0:s0 + 128],
                                         S_sb[j][:, :], start=(j == 0), stop=False)
                    nc.tensor.matmul(po[:, :], attnT[:, :], ui[:, :],
                                     start=False, stop=True)
                    otm = kb.scratch([128, 256])
                    nc.vector.tensor_scalar(out=otm[:, :], in0=po[:, :],
                                            scalar1=rqcol, scalar2=None,
                                            op0=OP.mult)
                    for j in range(2):
                        pds = kb.psum([128, 256])
                        nc.tensor.matmul(pds[:, :], kntm[:, 128 * j:128 * (j + 1)],
                                         ui[:, :], start=True, stop=True)
                        nc.vector.tensor_tensor(out=S_sb[j][:, :],
                                                in0=S_sb[j][:, :],
                                                in1=pds[:, :], op=OP.add)
                    pot = kb.psum([128, 256])
                    for j in range(2):
                        nc.tensor.matmul(pot[:, 128 * j:128 * (j + 1)],
                                         otm[:, 128 * j:128 * (j + 1)],
                                         iden[:, :], start=True, stop=True)
                    osb = kb.scratch([128, 256])
                    nc.scalar.copy(out=osb[:, :], in_=pot[:, :])
                    for j in range(2):
                        nc.sync.dma_start(
                            out=ODC[128 * j:128 * (j + 1), c0:c0 + 128],
                            in_=osb[:, 128 * j:128 * (j + 1)])
        kb.ctx.close()
    nc.compile()
    return nc


# ================= K3 (per time half) =================
def build_k3(p):
    kb = KB()
    nc = kb.nc
    HID16 = nc.dram_tensor("hid16", [L, HS], f16, kind="ExternalInput")
    VC = nc.dram_tensor("vc", [CH, L + 63], f32, kind="ExternalInput")
    FS = nc.dram_tensor("fs", [CH, L], f32, kind="ExternalInput")
    FL = nc.dram_tensor("fl", [CH, L], f32, kind="ExternalInput")
    OD0 = nc.dram_tensor("odc0", [256, L], f32, kind="ExternalInput")
    OD1 = nc.dram_tensor("odc1", [256, L], f32, kind="ExternalInput")
    WP = nc.dram_tensor("wpk3f", [1, W3TOT], f16, kind="ExternalInput")
    CP = nc.dram_tensor("cpk3", [1, C3TOT], f32, kind="ExternalInput")
    if p == 0:
        OUTP = nc.dram_tensor("outp0", [L // 2, HS], f32, kind="ExternalOutput")
    else:
        O0 = nc.dram_tensor("outp0", [L // 2, HS], f32, kind="ExternalInput")
        OUT = nc.dram_tensor("out", [L // 2, HS], f16, kind="ExternalOutput")
    with tile.TileContext(nc) as tc:
        kb.start(tc)
        if p == 1:
            outp_full = kb.dram.tile([L, HS], f32)
            outr = kb.dram.tile([L // 2, HS], f32)
            OUTP = outp_full[L // 2:L, :]
            nc.gpsimd.dma_start(outp_full[0:L // 2, :], O0[:, :])
        nc_ = nc
        ones = kb.ld2(CP, "ones", (128, 128), C3OFF["ones"])
        w1s = kb.ld2(CP, "w1sT", (16, HS), C3OFF["w1sT"], parts=32)
        b2s_t = kb.ld2(CP, "b2s", (4, 2), C3OFF["b2s"], parts=32)
        invt_t = kb.ld2(CP, "invt", (4, 2), C3OFF["invt"], parts=32)
        w2f = kb.ld3(CP, "w2T", 8, 4, C3OFF["w2T"])
        w2_t = kb.const.tile([128, 8, 4], bf16)
        nc.vector.tensor_copy(out=w2_t[:, :, :], in_=w2f[:, :, :])
        b1_t = kb.const.tile([128, 8], f32)
        nc.sync.dma_start(out=b1_t[:, :],
                          in_=_ap(CP, C3OFF["b1"], [[1, 128], [128, 8]]))
        normw_t = kb.const.tile([128, 2], f32)
        nc.sync.dma_start(out=normw_t[:, :],
                          in_=_ap(CP, C3OFF["normw"], [[1, 128], [128, 2]]))
        epsrms = kb.const.tile([128, 1], f32)
        nc.vector.memset(epsrms[:, :], RMS_EPS)
        w1 = kb.ld3(WP, "w1hT", 8, HS, W3OFF["w1hT"], f16)
        wo16 = kb.ld3(WP, "woT", 4, HS, W3OFF["woT"], f16)
        hidT = HID16[:, :].rearrange("a b -> b a")

        STATS_SRC = [("fs", FS, 0), ("fl", FL, 0), ("od", None, 0), ("v", VC, 63)]
        with tc.tile_pool(name="p7", bufs=2) as p7, \
             tc.tile_pool(name="p7one", bufs=1) as p7one, \
             tc.tile_pool(name="onp", bufs=2) as onp:
            for tbl in range(4):
                tb = 4 * p + tbl
                t0 = tb * 512
                tl0 = tbl * 512
                ht = []
                for k in range(8):
                    hh = p7.tile([128, 512], f16, tag=f"ht{k}", name=f"ht{k}")
                    nc.sync.dma_start(
                        out=hh[:, :], in_=hidT[128 * k:128 * (k + 1), t0:t0 + 512])
                    ht.append(hh)
                on_all = []
                for h in range(2):
                    r0 = 256 * h
                    odh = OD0 if h == 0 else OD1

                    def src_rows(ti, j):
                        nm, src, voff = STATS_SRC[ti]
                        if nm == "od":
                            return odh[128 * j:128 * (j + 1), t0:t0 + 512]
                        return src[r0 + 128 * j:r0 + 128 * (j + 1),
                                   voff + t0:voff + t0 + 512]

                    stats_blk = p7.tile([32, 512], f32, tag="statsblk",
                                        name="statsblk")
                    for ti in range(4):
                        bt = [p7.tile([128, 512], f32, tag=f"bt{j}", name=f"bt{j}")
                              for j in range(2)]
                        for j in range(2):
                            nc.sync.dma_start(out=bt[j][:, :], in_=src_rows(ti, j))
                        sq = [kb.scratch([128, 512]) for _ in range(2)]
                        ab = [kb.scratch([128, 512]) for _ in range(2)]
                        for j in range(2):
                            nc.scalar.activation(out=sq[j][:, :], in_=bt[j][:, :],
                                                 func=AF.Square)
                            nc.scalar.activation(out=ab[j][:, :], in_=bt[j][:, :],
                                                 func=AF.Abs)
                        psx = kb.psum([1, 512])
                        psq2 = kb.psum([1, 512])
                        psa = kb.psum([1, 512])
                        for j in range(2):
                            nc.tensor.matmul(psx[:, :], ones[:, 0:1], bt[j][:, :],
                                             start=(j == 0), stop=(j == 1))
                            nc.tensor.matmul(psq2[:, :], ones[:, 0:1], sq[j][:, :],
                                             start=(j == 0), stop=(j == 1))
                            nc.tensor.matmul(psa[:, :], ones[:, 0:1], ab[j][:, :],
                                             start=(j == 0), stop=(j == 1))
                        r = 4 * ti
                        fin = kb.scratch([1, 4, 512])
                        nc.scalar.activation(out=fin[:, 0, :], in_=psx[:, :],
                                             func=AF.Copy, scale=1.0 / 256.0)
                        m2 = kb.scratch([1, 512])
                        nc.vector.tensor_tensor(out=m2[:, :], in0=fin[:, 0, :],
                                                in1=fin[:, 0, :], op=OP.mult)
                        nc.vector.scalar_tensor_tensor(
                            out=fin[:, 1, :], in0=psq2[:, :], scalar=1.0 / 256.0,
                            in1=m2[:, :], op0=OP.mult, op1=OP.subtract)
                        nc.scalar.activation(out=fin[:, 2, :], in_=psa[:, :],
                                             func=AF.Copy, scale=1.0 / 256.0)
                        nc.scalar.activation(out=fin[:, 3, :], in_=psq2[:, :],
                                             func=AF.Sqrt)
                        nc.sync.dma_start(
                            out=stats_blk[r:r + 4, :],
                            in_=bass.AP(tensor=fin.tensor, offset=fin.offset,
                                        ap=[[512, 4], [1, 512]]))
                    # gate MLP
                    h1 = p7one.tile([128, 8, 512], bf16, tag="h1", name="h1")
                    for gm in range(8):
                        pg1 = kb.psum([128, 512])
                        for k in range(8):
                            nc.tensor.matmul(pg1[:, :],
                                             w1[:, k, 128 * gm:128 * (gm + 1)],
                                             ht[k][:, :], start=(k == 0),
                                             stop=False)
                        nc.tensor.matmul(pg1[:, :],
                                         w1s[0:16, 128 * gm:128 * (gm + 1)],
                                         stats_blk[0:16, :], start=False, stop=True)
                        nc.scalar.activation(out=h1[:, gm, :], in_=pg1[:, :],
                                             func=AF.Gelu_apprx_tanh,
                                             bias=b1_t[:, gm:gm + 1])
                    pl = kb.psum([4, 512])
                    for k in range(8):
                        nc.tensor.matmul(pl[:, :], w2_t[:, k, :], h1[:, k, :],
                                         start=(k == 0), stop=(k == 7))
                    el = kb.scratch([4, 512])
                    nc.scalar.activation(out=el[:, :], in_=pl[:, :], func=AF.Exp,
                                         scale=invt_t[0:4, h:h + 1],
                                         bias=b2s_t[0:4, h:h + 1])
                    pse = kb.psum([1, 512])
                    nc.tensor.matmul(pse[:, :], ones[0:4, 0:1], el[:, :],
                                     start=True, stop=True)
                    rec = kb.scratch([1, 512])
                    nc.vector.reciprocal(out=rec[:, :], in_=pse[:, :])
                    prr = kb.psum([4, 512])
                    nc.tensor.matmul(prr[:, :], ones[0:1, 0:4], rec[:, :],
                                     start=True, stop=True)
                    wgt = p7.tile([4, 512], f32, tag="wgtt", name="wgtt")
                    nc.vector.tensor_tensor(out=wgt[:, :], in0=el[:, :],
                                            in1=prr[:, :], op=OP.mult)
                    nc.vector.tensor_scalar(out=wgt[:, :], in0=wgt[:, :],
                                            scalar1=1.0 - 4.0 * EPS_FLOOR,
                                            scalar2=EPS_FLOOR, op0=OP.mult,
                                            op1=OP.add)
                    wbc = p7one.tile([128, 4, 512], f32, tag="wbc", name="wbc")
                    for cls in range(4):
                        wrow = kb.scratch([1, 512])
                        nc.sync.dma_start(out=wrow[:, :], in_=wgt[cls:cls + 1, :])
                        pwb = kb.psum([128, 512])
                        nc.tensor.matmul(pwb[:, :], ones[0:1, :], wrow[:, :],
                                         start=True, stop=True)
                        nc.scalar.copy(out=wbc[:, cls, :], in_=pwb[:, :])
                    ob = [p7one.tile([128, 512], f32, tag=f"ob{j}", name=f"ob{j}")
                          for j in range(2)]
                    osq = [kb.scratch([128, 512]) for _ in range(2)]
                    for cls in range(4):
                        for j in range(2):
                            bl = p7.tile([128, 512], f32, tag=f"bl{j}",
                                         name=f"bl{j}")
                            nc.sync.dma_start(out=bl[:, :], in_=src_rows(cls, j))
                            if cls == 0:
                                nc.vector.tensor_tensor(out=ob[j][:, :],
                                                        in0=bl[:, :],
                                                        in1=wbc[:, 0, :],
                                                        op=OP.mult)
                            else:
                                tbr = kb.scratch([128, 512])
                                nc.vector.tensor_tensor(out=tbr[:, :],
                                                        in0=bl[:, :],
                                                        in1=wbc[:, cls, :],
                                                        op=OP.mult)
                                nc.vector.tensor_tensor(out=ob[j][:, :],
                                                        in0=ob[j][:, :],
                                                        in1=tbr[:, :], op=OP.add)
                    for j in range(2):
                        nc.scalar.activation(out=osq[j][:, :], in_=ob[j][:, :],
                                             func=AF.Square)
                    pso = kb.psum([1, 512])
                    for j in range(2):
                        nc.tensor.matmul(pso[:, :], ones[:, 0:1], osq[j][:, :],
                                         start=(j == 0), stop=(j == 1))
                    srt = kb.scratch([1, 512])
                    nc.scalar.activation(out=srt[:, :], in_=pso[:, :],
                                         func=AF.Sqrt, scale=1.0 / 256.0,
                                         bias=epsrms[0:1, :])
                    rre = kb.scratch([1, 512])
                    nc.vector.reciprocal(out=rre[:, :], in_=srt[:, :])
                    prn = kb.psum([128, 512])
                    nc.tensor.matmul(prn[:, :], ones[0:1, :], rre[:, :],
                                     start=True, stop=True)
                    ons = []
                    for j in range(2):
                        on = onp.tile([128, 512], f16, tag=f"on{h}{j}",
                                      name=f"on{h}{j}")
                        nc.vector.scalar_tensor_tensor(
                            out=on[:, :], in0=ob[j][:, :],
                            scalar=normw_t[:, j:j + 1], in1=prn[:, :],
                            op0=OP.mult, op1=OP.mult)
                        ons.append(on)
                    on_all.append(ons)
                for m in range(8):
                    pw = kb.psum([128, 512])
                    for d in range(4):
                        nc.tensor.matmul(pw[:, :],
                                         wo16[:, d, 128 * m:128 * (m + 1)],
                                         on_all[d // 2][d % 2][:, :],
                                         start=(d == 0), stop=(d == 3))
                    owr = kb.scratch([128, 512])
                    nc.vector.tensor_copy(out=owr[:, :], in_=pw[:, :])
                    nc.sync.dma_start(
                        out=OUTP[tl0:tl0 + 512,
                                 128 * m:128 * (m + 1)].rearrange("a b -> b a"),
                        in_=owr[:, :])
        if p == 1:
            nc.gpsimd.collective_compute(
                "ReduceScatter", OP.add, replica_groups=PAIRS,
                ins=[outp_full.opt()], outs=[outr.opt()])
            with tc.tile_pool(name="p10", bufs=2) as p10:
                for r in range(16):
                    for half in range(2):
                        t = p10.tile([128, 512], f32, tag="oload", name="oload")
                        nc.sync.dma_start(
                            out=t[:, :],
                            in_=outr[128 * r:128 * (r + 1),
                                     512 * half:512 * (half + 1)])
                        t16 = p10.tile([128, 512], f16, tag="o16", name="o16")
                        nc.vector.tensor_copy(out=t16[:, :], in_=t[:, :])
                        nc.sync.dma_start(
                            out=OUT[128 * r:128 * (r + 1),
                                    512 * half:512 * (half + 1)],
                            in_=t16[:, :])
        kb.ctx.close()
    nc.compile()
    return nc


# ================= runner =================
def make_jit(nc, mesh):
    install_neuronx_cc_hook()
    pname = nc.partition_id_tensor.name if nc.partition_id_tensor else None
    in_names, out_names, out_avals = [], [], []
    for alloc in nc.m.functions[0].allocations:
        if not isinstance(alloc, mybir.MemoryLocationSet):
            continue
        name = alloc.memorylocations[0].name
        if alloc.kind == "ExternalInput":
            if name != pname:
                in_names.append(name)
        elif alloc.kind == "ExternalOutput":
            out_names.append(name)
            out_avals.append(jax.core.ShapedArray(
                tuple(alloc.tensor_shape), mybir.dt.np(alloc.dtype)))
    n_params = len(in_names)
    all_names = list(in_names)
    if pname is not None:
        all_names.append(pname)
    all_names = tuple(all_names)

    def _body(*args):
        operands = list(args)
        if pname is not None:
            operands.append(partition_id_tensor())
        outs = _bass_exec_p.bind(
            *operands, out_avals=tuple(out_avals), in_names=all_names,
            out_names=tuple(out_names), lowering_input_output_aliases=(),
            sim_require_finite=True, sim_require_nnan=True, nc=nc)
        return tuple(outs)

    P = PartitionSpec
    fn = jax.jit(
        shard_map(_body, mesh=mesh,
                  in_specs=(P("core"),) * n_params,
                  out_specs=(P("core"),) * len(out_names), check_rep=False),
        keep_unused=True)
    return fn, in_names, out_names, out_avals


class Chain:
    def __init__(self, ncs):
        self.mesh = Mesh(np.asarray(jax.devices()[:8]), ("core",))
        self.jits = [make_jit(nc, self.mesh) for nc in ncs]

    def run(self, host_inputs):
        """host_inputs: dict name -> np array (8*rows, cols). Returns bufs."""
        bufs = dict(host_inputs)
        for fn, in_names, out_names, out_avals in self.jits:
            args = [bufs[n] for n in in_names]
            outs = fn(*args)
            bufs.update(zip(out_names, outs))
        return bufs


_CHAIN = None


def get_chain():
    global _CHAIN
    if _CHAIN is None:
        ncs = [build_k1(), build_k2(0), build_k2(1), build_k3(0), build_k3(1)]
        _CHAIN = Chain(ncs)
    return _CHAIN


# ================= host packing =================
def pack_inputs(inputs):
    hidden = np.asarray(inputs["hidden_states"], np.float32)
    Wq = np.asarray(inputs["Wq"], np.float32)
    Wk = np.asarray(inputs["Wk"], np.float32)
    Wv = np.asarray(inputs["Wv"], np.float32)
    Wb = np.asarray(inputs["Wb"], np.float32)
    W1 = np.asarray(inputs["gate_W1"], np.float32)
    W2 = np.asarray(inputs["gate_W2"], np.float32)
    b1 = np.asarray(inputs["gate_b1"], np.float32)
    b2 = np.asarray(inputs["gate_b2"], np.float32)
    cpb = np.asarray(inputs["gate_copy_bias"], np.float32)
    ltp = np.asarray(inputs["gate_log_temp"], np.float32)
    Wo = np.asarray(inputs["Wo"], np.float32)
    normw = np.asarray(inputs["o_norm_w"], np.float32)
    cq = np.asarray(inputs["conv_q_w"], np.float32)
    ck = np.asarray(inputs["conv_k_w"], np.float32)
    cv = np.asarray(inputs["conv_v_w"], np.float32)
    firs = np.asarray(inputs["fir_short_filt"], np.float32).reshape(NH * DV, FIRS)
    firl = np.asarray(inputs["fir_long_filt"], np.float32).reshape(NH * DV, FIRL)

    iden = np.eye(128, dtype=np.float32)
    a = np.arange(128)
    negU = np.where(a[:, None] < a[None, :], -1.0, 0.0).astype(np.float32)
    uincl = np.where(a[:, None] <= a[None, :], 1.0, 0.0).astype(np.float32)
    onesm = np.ones((128, 128), np.float32)

    def pk(spec, offs, tot, parts, dt):
        buf = np.empty(tot, dt)
        for nm, shp in spec:
            buf[offs[nm]:offs[nm] + int(np.prod(shp))] = \
                parts[nm].astype(dt).ravel()
        return buf.reshape(1, -1)

    # per-variant (head-half) packs, built once each
    w1flat, w3flat, cpk1v, cpk3v = {}, {}, {}, {}
    for hl in range(2):
        rows = slice(512 * hl, 512 * (hl + 1))
        heads = [2 * hl, 2 * hl + 1]
        invt = np.exp(-ltp[heads])
        b2eff = np.stack([b2 + np.array([0, 0, 0, cpb[hh] * DECAY], np.float32)
                          for hh in heads], 1)
        w1flat[hl] = pk(WPK1, W1OFF, W1TOT,
                        {"wqT": Wq[rows].T, "wkT": Wk[rows].T,
                         "wvT": Wv[rows].T, "wbT": Wb[heads].T}, np.float16)
        w3flat[hl] = pk(WPK3, W3OFF, W3TOT,
                        {"w1hT": W1[:, :HS].T, "woT": Wo[:, rows].T}, np.float16)
        cpk1v[hl] = pk(CPK1, C1OFF, C1TOT,
                       {"convq": cq[rows], "convk": ck[rows],
                        "convv": cv[rows], "firs": firs[rows],
                        "firl": firl[rows]}, np.float32)
        cpk3v[hl] = pk(CPK3, C3OFF, C3TOT,
                       {"ones": onesm, "w1sT": W1[:, HS:HS + 16].T,
                        "w2T": W2.T, "b1": b1.reshape(HS, 1),
                        "normw": normw.reshape(DV, 1),
                        "b2s": b2eff * invt[None, :],
                        "invt": np.broadcast_to(invt[None, :], (4, 2))},
                       np.float32)
    cpk2v = pk(CPK2, C2OFF, C2TOT,
               {"iden": iden, "negU": negU, "uincl": uincl, "ones": onesm},
               np.float32)
    qtr, qtr3 = W1TOT // 4, W3TOT // 4
    per = {k: [] for k in ("hhalf", "wpk1", "cpk1", "cpk2", "wpk3q", "cpk3")}
    for c in range(8):
        b, hl = c // 2, c % 2
        per["hhalf"].append(np.ascontiguousarray(
            hidden[b, 2048 * hl:2048 * (hl + 1), :].astype(np.float16)))
        per["wpk1"].append(w1flat[hl][:, (c // 2) * qtr:(c // 2 + 1) * qtr])
        per["cpk1"].append(cpk1v[hl])
        per["cpk2"].append(cpk2v)
        per["wpk3q"].append(w3flat[hl][:, (c // 2) * qtr3:(c // 2 + 1) * qtr3])
        per["cpk3"].append(cpk3v[hl])
    return {k: np.concatenate(v, axis=0) for k, v in per.items()}


def unpack_output(out_global):
    o = np.asarray(out_global).reshape(8, 2048, HS)
    out = np.empty((B, L, HS), np.float32)
    for c in range(8):
        b, hl = c // 2, c % 2
        out[b, 2048 * hl:2048 * (hl + 1), :] = o[c]
    return out


def kernel(**inputs):
    chain = get_chain()
    host = pack_inputs(inputs)
    bufs = chain.run(host)
    return unpack_output(bufs["out"])


# ================= warm-load at import =================
def _warmup():
    chain = get_chain()
    host = {
        "hhalf": np.zeros((8 * 2048, HS), np.float16),
        "wpk1": np.zeros((8, W1TOT // 4), np.float16),
        "cpk1": np.zeros((8, C1TOT), np.float32),
        "cpk2": np.zeros((8, C2TOT), np.float32),
        "wpk3q": np.zeros((8, W3TOT // 4), np.float16),
        "cpk3": np.zeros((8, C3TOT), np.float32),
    }
    bufs = chain.run(host)
    np.asarray(bufs["out"])
    return chain


try:
    _warmup()
except Exception:
    _CHAIN = None  # fall back to building lazily inside kernel()
